# revision 30
# baseline (speedup 1.0000x reference)
"""Trainium2 Bass kernel for nn_BasicTransformerBlock (key-frame cross attention).

Reference computation (B=16 frames, S=1024, C=320, H=8 heads, D=40):
    q = x @ Wq.T ; k = x @ Wk.T ; v = x @ Wv.T
    k, v are taken from frame `kf` only and shared by every frame
    out = softmax(q k^T / sqrt(D)) v     (per frame, per head)
    y = out @ Wo.T + bo

Sharding: data-parallel over frames - 8 cores x 2 frames each. Every core
redundantly computes K/V from the key frame (cheap) so no collectives are
needed; outputs just concatenate.

Per-core design (all matmuls bf16 at full PE rate; psum accumulates f32):
  - Projections run "transposed": xT [C,S] tiles feed qT/kT [c_out_pad, S]
    (heads zero-padded D=40 -> 64 at 64-aligned partition offsets). V is
    projected as [t, 320] and repacked per head into v8 [t, {40 V | 23 zero
    | 1 ones}] bf16 blocks.
  - Scores st [t, s] = kT-slice.T @ qT (per head, per 128-row t-tile).
  - exp(score*scale) is SPLIT across two engines: the Scalar engine's Exp
    activation and a custom fused-DVE op EXPQ_ANT that evaluates
    ((c3 x + c2) x + c1) x + 1, squared twice (= exp to 4.6e-4 rel) in one
    8-stage DVE pass. Both write bf16 pt tiles. ~60/40 split balances
    Scalar vs Vector busy time.
  - PV runs TRANSPOSED: out[s-tile, 64] = pt-slice.T @ v8-block with pt as
    the stationary operand, so the matmul free dim is 64 (vs 1024 in the
    classic orientation) - PV drops from 54.6us to 27.3us of PE time. The
    ones column of v8 makes psum col 63 the softmax denominator per s-row,
    so normalization is one strided reciprocal [128,8] + one broadcast
    tensor_mul per head (per-partition scalars - no partition broadcast
    tricks needed).
  - The normalized [s, dp] tiles are flipped back to [dp, s] for the
    output projection by DMA-engine transposes (dma_start_transpose,
    [128,128] bf16 tiles) - the DMA engines are otherwise ~85% idle.
  - O-projection: y[c-chunk, s] = wo-chunk.T @ outT (+bias via Scalar
    Identity), DMAed out per chunk; host un-transposes.
  - psum: "big" ring 3x[128,1024] (scores double-buffered + proj/O-proj
    time-share, in allocation order - PE is strictly in-order) + "pv" ring
    2x[128,512]. All 8 banks.
"""

import os
import sys

import numpy as np

try:
    import concourse  # noqa: F401
except ImportError:  # pragma: no cover
    for _p in ("/opt/trn_rl_repo", os.path.dirname(os.path.abspath(__file__))):
        if os.path.isdir(os.path.join(_p, "concourse")):
            sys.path.insert(0, _p)
            break

import ml_dtypes  # noqa: E402

import concourse.dve_ops as dve_ops  # noqa: E402
import concourse.dve_spec as dve_spec  # noqa: E402
import concourse.mybir as mybir  # noqa: E402
import concourse.tile as tile  # noqa: E402
from concourse import bacc  # noqa: E402
from concourse import bass_utils  # noqa: E402
from concourse.dve_spec import C0, C1, C2, One, Spec, Src0, sq  # noqa: E402
from concourse.dve_uop import DveOpSpec  # noqa: E402

F32 = mybir.dt.float32
BF16 = mybir.dt.bfloat16
BF = ml_dtypes.bfloat16

S = 1024          # sequence length per frame
C = 320           # channels
H = 8             # heads
D = 40            # head dim
DP = 64           # padded head dim
CP = H * DP       # 512, padded channels
NCORES = 8
FPC = 2           # frames per core
SCALE = float(D) ** -0.5
CPAD = 384        # c_in padded to 3*128

CI = [(0, 128), (128, 128), (256, 64)]    # c_in chunks of 320
CO = [(0, 128), (128, 128), (256, 64)]    # c_out chunks of 320

# Of the 128 exp tiles per core, how many go to the Scalar engine (rest on
# the Vector engine's custom EXPQ op). Balances ACT vs DVE busy time.
ACT_TILES = 80

_NC_CACHE: dict = {}
LAST_RESULTS = None  # set by _run for test harness introspection


# --- custom DVE op: exp via deg-3 poly squared twice ------------------------

def _fit_exp_coefs(r=8.8, scale=SCALE):
    """exp(scale*x) ~= (((c3 x + c2) x + c1) x + 1)^4-ish: inner fits
    exp(scale*x/4) with relative-error weighting; max rel err ~4.6e-4 on
    |x| <= r (raw scores here are within +-8.4)."""
    k = np.arange(4000)
    xs = r * np.cos(np.pi * (k + 0.5) / 4000)
    w = 1.0 / np.exp(scale * xs / 4)
    A = np.stack([xs**p for p in range(1, 4)], 1)
    y = np.exp(scale * xs / 4) - 1.0
    c, *_ = np.linalg.lstsq(A * w[:, None], y * w, rcond=None)
    return c  # [c1, c2, c3]


EXP_COEF = _fit_exp_coefs()


def _register_expq():
    name = "EXPQ_ANT"
    for op in dve_ops.OPS:
        if op.name == name:
            return op
    x = Src0
    inner = ((C0 * x + C1) * x + C2) * x + One
    body = sq(sq(inner))

    def ref(in0, in1, s0, s1, imm2):
        xx = in0.astype(np.float32)
        p = ((s0 * xx + s1) * xx + imm2) * xx + 1.0
        return (p * p) ** 2

    spec = Spec(body=body, reference=ref)
    opcode = max(dve_ops._SUB_OPCODE_FOR_NAME.values()) + 1
    assert opcode < 0x20
    dve_ops._SUB_OPCODE_FOR_NAME[name] = opcode
    shas = {}
    for ver in ("v3", "v4"):
        s = DveOpSpec(
            name=name,
            opcode=opcode,
            uops=dve_spec.lower(spec, ver=ver),
            rd1_en=dve_spec._has_src1(spec),
        )
        shas[ver] = s.sha(ver)
    op = dve_ops.DveOp(name, spec, subdim=False, uops_sha=shas)
    dve_ops.OPS.append(op)
    dve_ops.CUSTOM_DVE_SPECS[name] = spec
    return op


EXPQ = _register_expq()


def _build(loop_n: int = 1):
    nc = bacc.Bacc("TRN2", target_bir_lowering=False, debug=False)

    xt0 = nc.dram_tensor("xt0", [CPAD, S], BF16, kind="ExternalInput")
    xtf = nc.dram_tensor("xtf", [FPC, CPAD, S], BF16, kind="ExternalInput")
    wkq = nc.dram_tensor("wkq", [CPAD, 2 * CP], BF16, kind="ExternalInput")
    whot = nc.dram_tensor("whot", [CPAD, 256], BF16, kind="ExternalInput")
    wvp = nc.dram_tensor("wvp", [CPAD, C], BF16, kind="ExternalInput")
    wo = nc.dram_tensor("wo", [CP, C], BF16, kind="ExternalInput")
    bo = nc.dram_tensor("bo", [CPAD], F32, kind="ExternalInput")
    ident = nc.dram_tensor("ident", [128, 128], BF16, kind="ExternalInput")
    yt = nc.dram_tensor("yt", [FPC, C, S], BF16, kind="ExternalOutput")

    c3f, c2f, c1f = float(EXP_COEF[2]), float(EXP_COEF[1]), float(EXP_COEF[0])

    with tile.TileContext(nc) as tc:
        with (
            tc.tile_pool(name="pconst", bufs=1) as pconst,
            tc.tile_pool(name="pqk", bufs=1) as pqk,
            tc.tile_pool(name="pvs", bufs=1) as pvs,
            tc.tile_pool(name="ppt", bufs=6) as ppt,
            tc.tile_pool(name="pnorm", bufs=3) as pnorm,
            tc.tile_pool(name="pot", bufs=1) as pot,
            tc.tile_pool(name="prc", bufs=4) as prc,
            tc.tile_pool(name="py", bufs=1) as py,
            tc.tile_pool(name="psb", bufs=3, space="PSUM") as psb,
            tc.tile_pool(name="psv", bufs=2, space="PSUM") as psv,
        ):
          for it in range(loop_n):
            P = f"{it}_"

            kT8 = [pqk.tile([128, S], BF16, name=f"{P}kT{m}", tag=f"kT{m}") for m in range(4)]
            qT8 = [
                [pqk.tile([128, S], BF16, name=f"{P}qT{f}_{m}", tag=f"qT{f}_{m}") for m in range(4)]
                for f in range(FPC)
            ]
            v8 = [pvs.tile([128, H * DP], BF16, name=f"{P}v{tt}", tag=f"v{tt}") for tt in range(8)]
            outT = [
                [pot.tile([128, S], BF16, name=f"{P}oT{f}_{hp}", tag=f"oT{f}_{hp}") for hp in range(4)]
                for f in range(FPC)
            ]

            # static v8 init: zero pad cols + ones col per head
            for tt in range(8):
                vv = v8[tt][:].rearrange("p (h c) -> p h c", c=DP)
                nc.gpsimd.memset(vv[:, :, D:DP], 0.0)
                nc.gpsimd.memset(vv[:, :, DP - 1:DP], 1.0)

            exp_ctr = [0]

            def emit_exp(pt_t, st_t, tt):
                i = exp_ctr[0]
                exp_ctr[0] += 1
                on_act = ((i + 1) * ACT_TILES) // 128 > (i * ACT_TILES) // 128
                if on_act:
                    nc.scalar.activation(
                        pt_t[:], st_t[:], mybir.ActivationFunctionType.Exp, scale=SCALE
                    )
                else:
                    nc.vector._custom_dve(
                        EXPQ, out=pt_t[:], in0=st_t[:], s0=c3f, s1=c2f, imm2=c1f
                    )

            def one_dma_tile(pool, nm, dram_ap, width):
                """[128, 3*width] bf16 tile <- [384, width] dram via one DMA."""
                t = pool.tile([128, 3 * width], BF16, name=f"{P}{nm}", tag=nm)
                nc.sync.dma_start(
                    t[:].rearrange("p (c w) -> p c w", w=width),
                    dram_ap.rearrange("(c p) w -> p c w", p=128),
                )
                return t, [t[0:cn, ci * width:ci * width + width] for ci, (cs, cn) in enumerate(CI)]

            def proj_full(dst, w_tiles, x_tiles, m):
                """dst[m] [128, S] bf16 <- (w m-chunk).T @ x, one psum pass."""
                ps = psb.tile([128, S], F32, name=f"{P}pp{dst[m].name}", tag="big")
                for sh in range(2):
                    for ci in range(3):
                        nc.tensor.matmul(
                            ps[:, sh * 512:(sh + 1) * 512],
                            w_tiles[ci][:, m * 128:(m + 1) * 128],
                            x_tiles[ci][:, sh * 512:(sh + 1) * 512],
                            start=(ci == 0),
                            stop=(ci == 2),
                        )
                nc.vector.tensor_copy(dst[m][:], ps[:])

            def scores_exp(f, hp, tt):
                """Scores + exp for one (unit, t-tile) step; returns pt pair."""
                pts = [None, None]
                for par in range(2):
                    st = psb.tile([128, S], F32, name=f"{P}st{f}{hp}{tt}{par}", tag="big")
                    hl = par * DP
                    for sh in range(2):
                        nc.tensor.matmul(
                            st[:, sh * 512:(sh + 1) * 512],
                            kT8[hp][hl:hl + DP, tt * 128:(tt + 1) * 128],
                            qT8[f][hp][hl:hl + DP, sh * 512:(sh + 1) * 512],
                            start=True,
                            stop=True,
                        )
                    pt = ppt.tile([128, S], BF16, name=f"{P}pt{f}{hp}{tt}{par}", tag="pt")
                    emit_exp(pt, st, tt)
                    pts[par] = pt
                return pts

            def pv_step(hp, tt, pv, pts):
                for par in range(2):
                    h = hp * 2 + par
                    for si in range(8):
                        # psum pending-zero is bank-granular (2KB): one start
                        # marks the whole bank; each si's first write then
                        # zero-fills its own region
                        nc.tensor.matmul(
                            pv[par][:, si * DP:(si + 1) * DP],
                            pts[par][:, si * 128:(si + 1) * 128],
                            v8[tt][:, h * DP:(h + 1) * DP],
                            start=(tt == 0 and si == 0),
                            stop=(tt == 7),
                        )

            def finish_unit(f, hp, pv, pe_flip=False):
                """normalize (psum col 63 of each si-block is the softmax
                denominator) and flip [s, dp] -> [dp, s]. Mid-stream units
                flip on the (idle) DMA engines; the last unit flips on PE
                (is_transpose matmuls) to keep the tail off the serialized
                hwdge queue."""
                norm = pnorm.tile([128, S], BF16, name=f"{P}nm{f}{hp}", tag="nm")
                normv = norm[:].rearrange("p (si c) -> p si c", c=128)
                for par in range(2):
                    pvv = pv[par][:].rearrange("p (si c) -> p si c", c=DP)
                    rc = prc.tile([128, 8], F32, name=f"{P}rc{f}{hp}{par}", tag="rc")
                    nc.vector.reciprocal(rc[:], pvv[:, :, DP - 1])
                    nc.vector.tensor_mul(
                        normv[:, :, par * DP:(par + 1) * DP],
                        pvv[:],
                        rc[:].unsqueeze(-1).broadcast_to([128, 8, DP]),
                    )
                if pe_flip:
                    trp = psv.tile([128, S], BF16, name=f"{P}trp{f}{hp}", tag="pv")
                    for si in range(8):
                        nc.tensor.matmul(
                            trp[:, si * 128:(si + 1) * 128],
                            norm[:, si * 128:(si + 1) * 128],
                            ident_sb[:],
                            is_transpose=True,
                            start=True,
                            stop=True,
                        )
                    nc.vector.tensor_copy(outT[f][hp][:], trp[:])
                else:
                    for si in range(8):
                        nc.sync.dma_start_transpose(
                            outT[f][hp][:, si * 128:(si + 1) * 128],
                            norm[:, si * 128:(si + 1) * 128],
                        )

            def vproj(tt):
                ps = psb.tile([128, S], F32, name=f"{P}ppv{tt}", tag="big")
                for ci in range(3):
                    nc.tensor.matmul(
                        ps[:, 0:C],
                        x0_sb[ci][:, tt * 128:(tt + 1) * 128],
                        wv_sb[ci][:],
                        start=(ci == 0),
                        stop=(ci == 2),
                    )
                vv = v8[tt][:].rearrange("p (h c) -> p h c", c=DP)
                nc.vector.tensor_copy(
                    vv[:, :, 0:D],
                    ps[:, 0:C].rearrange("p (h c) -> p h c", c=D),
                )

            def oproj_mm(f, m, ps, sh, cps=range(4)):
                cos, cn = CO[m]
                for cp in cps:
                    nc.tensor.matmul(
                        ps[:, sh * 512:(sh + 1) * 512],
                        wo_sb[cp][:, cos:cos + cn],
                        outT[f][cp][:, sh * 512:(sh + 1) * 512],
                        start=(cp == 0),
                        stop=(cp == 3),
                    )

            def oproj_fin(f, m, ps, last=False):
                cos, cn = CO[m]
                y_sb = py.tile([cn, S], BF16, name=f"{P}y{f}{m}", tag=f"y{m}")
                # mid-stream y DMAs go via the (idle) gpsimd software DGE so
                # the SP hwdge queue stays free for the outT transposes; the
                # tail ones use the (by then free) fast hwdge path, split in
                # halves so the first DMA overlaps the second bias
                eng = nc.sync if last else nc.gpsimd
                for sh in range(2) if last else (slice(None),):
                    sl = (
                        slice(sh * 512, (sh + 1) * 512)
                        if isinstance(sh, int)
                        else sh
                    )
                    nc.scalar.activation(
                        y_sb[:, sl], ps[:, sl],
                        mybir.ActivationFunctionType.Identity, bias=bo_sb[m][:],
                    )
                    eng.dma_start(yt.ap()[f, cos:cos + cn, sl], y_sb[:, sl])

            def oproj(f, m):
                cos, cn = CO[m]
                ps = psb.tile([cn, S], F32, name=f"{P}oy{f}{m}", tag="big")
                for sh in range(2):
                    oproj_mm(f, m, ps, sh)
                oproj_fin(f, m, ps)

            # ---- input DMAs (hot-path order; x tiles chunked so the hot
            # ---- projections start as soon as each c_in chunk lands)
            whot_t = pconst.tile([128, 3 * 256], BF16, name=f"{P}whota", tag="whota")
            whot_ap3 = whot.ap().rearrange("(c p) w -> p c w", p=128)
            whot_tv = whot_t[:].rearrange("p (c w) -> p c w", w=256)
            whot_v = [
                whot_t[0:cn, ci * 256:ci * 256 + 256] for ci, (cs, cn) in enumerate(CI)
            ]
            whot_k = [t[:, 0:128] for t in whot_v]
            whot_q = [t[:, 128:256] for t in whot_v]

            def chunked_x_dma(nm, dram_ap):
                t = pconst.tile([128, 3 * S], BF16, name=f"{P}{nm}", tag=nm)
                for ci in range(3):
                    nc.sync.dma_start(
                        t[:, ci * S:(ci + 1) * S],
                        dram_ap[ci * 128:(ci + 1) * 128, :],
                    )
                return t, [t[0:cn, ci * S:ci * S + S] for ci, (cs, cn) in enumerate(CI)]

            nc.sync.dma_start(whot_tv[:, :, 0:128], whot_ap3[:, :, 0:128])
            _, x0_sb = chunked_x_dma("x0a", xt0.ap())
            nc.sync.dma_start(whot_tv[:, :, 128:256], whot_ap3[:, :, 128:256])
            xf_sb = [None, None]
            _, xf_sb[0] = chunked_x_dma("xfa0", xtf.ap()[0])

            proj_full(kT8, whot_k, x0_sb, 0)
            proj_full(qT8[0], whot_q, xf_sb[0], 0)

            _, wv_sb = one_dma_tile(pconst, "wva", wvp.ap(), C)
            wkq_t = pconst.tile([128, 3 * 2 * CP], BF16, name=f"{P}wkqa", tag="wkqa")
            wkq_ap3 = wkq.ap().rearrange("(c p) w -> p c w", p=128)
            wkq_tv = wkq_t[:].rearrange("p (c w) -> p c w", w=2 * CP)
            nc.sync.dma_start(wkq_tv[:, :, 0:CP], wkq_ap3[:, :, 0:CP])
            nc.sync.dma_start(wkq_tv[:, :, CP:2 * CP], wkq_ap3[:, :, CP:2 * CP])
            wkq_v = [
                wkq_t[0:cn, ci * 2 * CP:ci * 2 * CP + 2 * CP]
                for ci, (cs, cn) in enumerate(CI)
            ]
            wk_sb = [t[:, 0:CP] for t in wkq_v]
            wq_sb = [t[:, CP:2 * CP] for t in wkq_v]
            _, xf_sb[1] = chunked_x_dma("xfa1", xtf.ap()[1])
            wo_all = pconst.tile([128, 4 * C], BF16, name=f"{P}wo", tag="wo")
            wo_sb = [wo_all[:, cp * C:(cp + 1) * C] for cp in range(4)]
            nc.sync.dma_start(
                wo_all[:].rearrange("p (cp c) -> p cp c", c=C),
                wo.ap().rearrange("(cp p) c -> p cp c", p=128),
            )
            bo_all = pconst.tile([128, 3], F32, name=f"{P}bo", tag="bo")
            nc.sync.dma_start(bo_all[:], bo.ap().rearrange("(c p) -> p c", p=128))
            bo_sb = [bo_all[0:cn, m:m + 1] for m, (cs, cn) in enumerate(CO)]
            ident_sb = pconst.tile([128, 128], BF16, name=f"{P}ident", tag="ident")
            nc.sync.dma_start(ident_sb[:], ident.ap())

            def pg(dst, w_t, x_t, ms):
                return [
                    (lambda m=m: proj_full(dst, w_t, x_t, m)) for m in ms
                ]

            # ---- globally software-pipelined attention: PV lags scores/exp
            # ---- by one step ACROSS unit boundaries so PE never restarts
            UNITS = [(0, 0), (0, 1), (0, 2), (0, 3), (1, 0), (1, 1), (1, 2), (1, 3)]
            extras_by_unit = [
                pg(kT8, wk_sb, x0_sb, [1]) + pg(qT8[0], wq_sb, xf_sb[0], [1]),
                pg(kT8, wk_sb, x0_sb, [2]) + pg(qT8[0], wq_sb, xf_sb[0], [2]),
                pg(kT8, wk_sb, x0_sb, [3]) + pg(qT8[0], wq_sb, xf_sb[0], [3]),
                pg(qT8[1], wq_sb, xf_sb[1], [0, 1]),
                pg(qT8[1], wq_sb, xf_sb[1], [2, 3]),
                [lambda: oproj(0, 0), lambda: oproj(0, 1)],
                [lambda: oproj(0, 2)],
                [],
            ]
            # PV lags scores/exp by LAG steps so the PV group's exp dependency
            # is already satisfied at dispatch time (the PE sequencer is
            # in-order with a shallow wait queue - a blocked head instruction
            # stalls dispatch of everything behind it)
            NG = len(UNITS) * 8
            LAG = 2
            pts_hist: dict = {}
            pv_cur = None
            oy = None
            for g in range(NG + LAG):
                gp = g - LAG
                if gp >= 0:
                    up, ttp = divmod(gp, 8)
                    fp, hpp = UNITS[up]
                    if ttp == 0:
                        pv_cur = [
                            psv.tile([128, 512], F32, name=f"{P}pv{fp}{hpp}{par}", tag="pv")
                            for par in range(2)
                        ]
                    pv_step(hpp, ttp, pv_cur, pts_hist.pop(gp))
                    if ttp % 2 == 1 and extras_by_unit[up]:
                        extras_by_unit[up].pop(0)()
                    if ttp == 7:
                        last = up == len(UNITS) - 1
                        if last:
                            # pre-issue the frame-1 O-proj partials that only
                            # need outT[1][0..2]; they hide the last unit's
                            # normalize + flip latency
                            oy = [
                                psb.tile([CO[m][1], S], F32, name=f"{P}oy1{m}", tag="big")
                                for m in range(3)
                            ]
                            for m in range(3):
                                for sh in range(2):
                                    oproj_mm(1, m, oy[m], sh, cps=range(3))
                        finish_unit(fp, hpp, pv_cur, pe_flip=last)
                if g < NG:
                    u, tt = divmod(g, 8)
                    pts_hist[g] = scores_exp(*UNITS[u], tt)
                    if g < 8:
                        vproj(g)
            for m in range(3):
                for sh in range(2):
                    oproj_mm(1, m, oy[m], sh, cps=[3])
                oproj_fin(1, m, oy[m], last=True)

    nc.compile()
    return nc


def _get_nc(loop_n: int = 1):
    if loop_n not in _NC_CACHE:
        _NC_CACHE[loop_n] = _build(loop_n)
    return _NC_CACHE[loop_n]


def _pad_heads_cols(wT: np.ndarray) -> np.ndarray:
    """[C, C] (c_in, c_out) -> [C, CP] with each head's 40 cols at h*64."""
    out = np.zeros((C, CP), np.float32)
    out.reshape(C, H, DP)[:, :, :D] = wT.reshape(C, H, D)
    return out


def _prep_inputs(hidden_states, Wq, Wk, Wv, Wo, bo, video_length, k):
    hidden_states = np.asarray(hidden_states, dtype=np.float32)
    B = hidden_states.shape[0]
    assert hidden_states.shape == (B, S, C), hidden_states.shape
    assert B == NCORES * FPC, B
    kf = int(k)
    vl = int(video_length)
    b = B // vl
    assert b == 1, "kernel specialized for batch 1 (b*video_length == B)"

    xT = np.zeros((B, CPAD, S), np.float32)
    xT[:, :C, :] = hidden_states.transpose(0, 2, 1)
    xT = xT.astype(BF)
    wk_p = _pad_heads_cols(np.asarray(Wk, np.float32).T)
    wq_p = _pad_heads_cols(np.asarray(Wq, np.float32).T)
    wkq_p = np.zeros((CPAD, 2 * CP), np.float32)
    wkq_p[:C] = np.concatenate([wk_p, wq_p], axis=1)
    whot_p = np.zeros((CPAD, 256), np.float32)
    whot_p[:C] = np.concatenate([wk_p[:, 0:128], wq_p[:, 0:128]], axis=1)
    wv_p = np.zeros((CPAD, C), np.float32)
    wv_p[:C] = np.asarray(Wv, np.float32).T
    # WoT padded rows: row h*64+j = Wo[:, h*40+j]; pad rows (incl. the ones/
    # denominator row 63) are zero
    wo_p = np.zeros((CP, C), np.float32)
    wo_p.reshape(H, DP, C)[:, :D, :] = np.asarray(Wo, np.float32).T.reshape(H, D, C)
    bo_f = np.zeros(CPAD, np.float32)
    bo_f[:C] = np.asarray(bo, np.float32)

    xt0 = np.ascontiguousarray(xT[kf])
    wkq_b = wkq_p.astype(BF)
    whot_b = whot_p.astype(BF)
    wv_b = wv_p.astype(BF)
    wo_b = wo_p.astype(BF)
    ident_b = np.eye(128, dtype=np.float32).astype(BF)
    in_maps = []
    for c in range(NCORES):
        in_maps.append(
            {
                "xt0": xt0,
                "xtf": np.ascontiguousarray(xT[c * FPC:(c + 1) * FPC]),
                "wkq": wkq_b,
                "whot": whot_b,
                "wvp": wv_b,
                "wo": wo_b,
                "bo": bo_f,
                "ident": ident_b,
            }
        )
    return in_maps


def _run(inputs: dict, loop_n: int = 1):
    global LAST_RESULTS
    nc = _get_nc(loop_n)
    in_maps = _prep_inputs(**inputs)
    last_exc = None
    for _attempt in range(3):
        try:
            res = bass_utils.run_bass_kernel_spmd(nc, in_maps, core_ids=list(range(NCORES)))
            break
        except Exception as e:  # transient NRT/axon device hiccups
            last_exc = e
            import time as _time
            _time.sleep(2.0)
    else:
        raise last_exc
    LAST_RESULTS = res
    B = NCORES * FPC
    y = np.empty((B, S, C), np.float32)
    for c in range(NCORES):
        y[c * FPC:(c + 1) * FPC] = (
            res.results[c]["yt"].astype(np.float32).transpose(0, 2, 1)
        )
    return y


def kernel(hidden_states, Wq, Wk, Wv, Wo, bo, video_length, k):
    return _run(
        dict(
            hidden_states=hidden_states,
            Wq=Wq,
            Wk=Wk,
            Wv=Wv,
            Wo=Wo,
            bo=bo,
            video_length=video_length,
            k=k,
        )
    )


# revision 32
# speedup vs baseline: 1.0694x; 1.0694x over previous
"""Trainium2 Bass kernel for nn_BasicTransformerBlock (key-frame cross attention).

Reference computation (B=16 frames, S=1024, C=320, H=8 heads, D=40):
    q = x @ Wq.T ; k = x @ Wk.T ; v = x @ Wv.T
    k, v are taken from frame `kf` only and shared by every frame
    out = softmax(q k^T / sqrt(D)) v     (per frame, per head)
    y = out @ Wo.T + bo

Sharding: data-parallel over frames - 8 cores x 2 frames each. Every core
redundantly computes K/V from the key frame (cheap) so no collectives are
needed; outputs just concatenate.

Per-core design (all matmuls bf16 at full PE rate; psum accumulates f32):
  - Projections run "transposed": xT [C,S] tiles feed qT/kT [c_out_pad, S]
    (heads zero-padded D=40 -> 64 at 64-aligned partition offsets). V is
    projected as [t, 320] and repacked per head into v8 [t, {40 V | 23 zero
    | 1 ones}] bf16 blocks.
  - Scores st [t, s] = kT-slice.T @ qT (per head, per 128-row t-tile).
  - exp(score*scale) is SPLIT across two engines: the Scalar engine's Exp
    activation and a custom fused-DVE op EXPQ_ANT that evaluates
    ((c3 x + c2) x + c1) x + 1, squared twice (= exp to 4.6e-4 rel) in one
    8-stage DVE pass. Both write bf16 pt tiles. ~60/40 split balances
    Scalar vs Vector busy time.
  - PV runs TRANSPOSED: out[s-tile, 64] = pt-slice.T @ v8-block with pt as
    the stationary operand, so the matmul free dim is 64 (vs 1024 in the
    classic orientation) - PV drops from 54.6us to 27.3us of PE time. The
    ones column of v8 makes psum col 63 the softmax denominator per s-row,
    so normalization is one strided reciprocal [128,8] + one broadcast
    tensor_mul per head (per-partition scalars - no partition broadcast
    tricks needed).
  - The normalized [s, dp] tiles are flipped back to [dp, s] for the
    output projection by DMA-engine transposes (dma_start_transpose,
    [128,128] bf16 tiles) - the DMA engines are otherwise ~85% idle.
  - O-projection: y[c-chunk, s] = wo-chunk.T @ outT (+bias via Scalar
    Identity), DMAed out per chunk; host un-transposes.
  - psum: "big" ring 3x[128,1024] (scores double-buffered + proj/O-proj
    time-share, in allocation order - PE is strictly in-order) + "pv" ring
    2x[128,512]. All 8 banks.
"""

import os
import sys

import numpy as np

try:
    import concourse  # noqa: F401
except ImportError:  # pragma: no cover
    for _p in ("/opt/trn_rl_repo", os.path.dirname(os.path.abspath(__file__))):
        if os.path.isdir(os.path.join(_p, "concourse")):
            sys.path.insert(0, _p)
            break

import ml_dtypes  # noqa: E402

import concourse.dve_ops as dve_ops  # noqa: E402
import concourse.dve_spec as dve_spec  # noqa: E402
import concourse.mybir as mybir  # noqa: E402
import concourse.tile as tile  # noqa: E402
from concourse import bacc  # noqa: E402
from concourse import bass_utils  # noqa: E402
from concourse.dve_spec import C0, C1, C2, One, Spec, Src0, sq  # noqa: E402
from concourse.dve_uop import DveOpSpec  # noqa: E402

F32 = mybir.dt.float32
BF16 = mybir.dt.bfloat16
BF = ml_dtypes.bfloat16

S = 1024          # sequence length per frame
C = 320           # channels
H = 8             # heads
D = 40            # head dim
DP = 64           # padded head dim
CP = H * DP       # 512, padded channels
NCORES = 8
FPC = 2           # frames per core
SCALE = float(D) ** -0.5
CPAD = 384        # c_in padded to 3*128

CI = [(0, 128), (128, 128), (256, 64)]    # c_in chunks of 320
CO = [(0, 128), (128, 128), (256, 64)]    # c_out chunks of 320

# Of the 128 exp tiles per core, how many go to the Scalar engine (rest on
# the Vector engine's custom EXPQ op). Balances ACT vs DVE busy time.
ACT_TILES = 80

_NC_CACHE: dict = {}
LAST_RESULTS = None  # set by _run for test harness introspection


# --- custom DVE op: exp via deg-3 poly squared twice ------------------------

def _fit_exp_coefs(r=8.8, scale=SCALE):
    """exp(scale*x) ~= (((c3 x + c2) x + c1) x + 1)^4-ish: inner fits
    exp(scale*x/4) with relative-error weighting; max rel err ~4.6e-4 on
    |x| <= r (raw scores here are within +-8.4)."""
    k = np.arange(4000)
    xs = r * np.cos(np.pi * (k + 0.5) / 4000)
    w = 1.0 / np.exp(scale * xs / 4)
    A = np.stack([xs**p for p in range(1, 4)], 1)
    y = np.exp(scale * xs / 4) - 1.0
    c, *_ = np.linalg.lstsq(A * w[:, None], y * w, rcond=None)
    return c  # [c1, c2, c3]


EXP_COEF = _fit_exp_coefs()


def _register_expq():
    name = "EXPQ_ANT"
    for op in dve_ops.OPS:
        if op.name == name:
            return op
    x = Src0
    inner = ((C0 * x + C1) * x + C2) * x + One
    body = sq(sq(inner))

    def ref(in0, in1, s0, s1, imm2):
        xx = in0.astype(np.float32)
        p = ((s0 * xx + s1) * xx + imm2) * xx + 1.0
        return (p * p) ** 2

    spec = Spec(body=body, reference=ref)
    opcode = max(dve_ops._SUB_OPCODE_FOR_NAME.values()) + 1
    assert opcode < 0x20
    dve_ops._SUB_OPCODE_FOR_NAME[name] = opcode
    shas = {}
    for ver in ("v3", "v4"):
        s = DveOpSpec(
            name=name,
            opcode=opcode,
            uops=dve_spec.lower(spec, ver=ver),
            rd1_en=dve_spec._has_src1(spec),
        )
        shas[ver] = s.sha(ver)
    op = dve_ops.DveOp(name, spec, subdim=False, uops_sha=shas)
    dve_ops.OPS.append(op)
    dve_ops.CUSTOM_DVE_SPECS[name] = spec
    return op


EXPQ = _register_expq()


def _build(loop_n: int = 1):
    nc = bacc.Bacc("TRN2", target_bir_lowering=False, debug=False)

    xt0 = nc.dram_tensor("xt0", [CPAD, S], BF16, kind="ExternalInput")
    xtf = nc.dram_tensor("xtf", [FPC, CPAD, S], BF16, kind="ExternalInput")
    wkq = nc.dram_tensor("wkq", [CPAD, 2 * CP], BF16, kind="ExternalInput")
    whot = nc.dram_tensor("whot", [CPAD, 256], BF16, kind="ExternalInput")
    wvp = nc.dram_tensor("wvp", [CPAD, C], BF16, kind="ExternalInput")
    wo = nc.dram_tensor("wo", [CP, C], BF16, kind="ExternalInput")
    bo = nc.dram_tensor("bo", [CPAD], F32, kind="ExternalInput")
    ident = nc.dram_tensor("ident", [128, 128], BF16, kind="ExternalInput")
    yt = nc.dram_tensor("yt", [FPC, C, S], BF16, kind="ExternalOutput")

    c3f, c2f, c1f = float(EXP_COEF[2]), float(EXP_COEF[1]), float(EXP_COEF[0])

    with tile.TileContext(nc) as tc:
        with (
            tc.tile_pool(name="pconst", bufs=1) as pconst,
            tc.tile_pool(name="pqk", bufs=1) as pqk,
            tc.tile_pool(name="pvs", bufs=1) as pvs,
            tc.tile_pool(name="ppt", bufs=6) as ppt,
            tc.tile_pool(name="pnorm", bufs=3) as pnorm,
            tc.tile_pool(name="pot", bufs=1) as pot,
            tc.tile_pool(name="prc", bufs=4) as prc,
            tc.tile_pool(name="py", bufs=1) as py,
            tc.tile_pool(name="psb", bufs=3, space="PSUM") as psb,
            tc.tile_pool(name="psv", bufs=2, space="PSUM") as psv,
        ):
          for it in range(loop_n):
            P = f"{it}_"

            kT8 = [pqk.tile([128, S], BF16, name=f"{P}kT{m}", tag=f"kT{m}") for m in range(4)]
            qT8 = [
                [pqk.tile([128, S], BF16, name=f"{P}qT{f}_{m}", tag=f"qT{f}_{m}") for m in range(4)]
                for f in range(FPC)
            ]
            v8 = [pvs.tile([128, H * DP], BF16, name=f"{P}v{tt}", tag=f"v{tt}") for tt in range(8)]
            outT = [
                [pot.tile([128, S], BF16, name=f"{P}oT{f}_{hp}", tag=f"oT{f}_{hp}") for hp in range(4)]
                for f in range(FPC)
            ]

            # static v8 init: zero pad cols + ones col per head
            for tt in range(8):
                vv = v8[tt][:].rearrange("p (h c) -> p h c", c=DP)
                nc.gpsimd.memset(vv[:, :, D:DP], 0.0)
                nc.gpsimd.memset(vv[:, :, DP - 1:DP], 1.0)

            exp_ctr = [0]

            def emit_exp(pt_t, st_t, tt):
                i = exp_ctr[0]
                exp_ctr[0] += 1
                on_act = ((i + 1) * ACT_TILES) // 128 > (i * ACT_TILES) // 128
                if on_act:
                    nc.scalar.activation(
                        pt_t[:], st_t[:], mybir.ActivationFunctionType.Exp, scale=SCALE
                    )
                else:
                    nc.vector._custom_dve(
                        EXPQ, out=pt_t[:], in0=st_t[:], s0=c3f, s1=c2f, imm2=c1f
                    )

            def one_dma_tile(pool, nm, dram_ap, width):
                """[128, 3*width] bf16 tile <- [384, width] dram via one DMA."""
                t = pool.tile([128, 3 * width], BF16, name=f"{P}{nm}", tag=nm)
                nc.sync.dma_start(
                    t[:].rearrange("p (c w) -> p c w", w=width),
                    dram_ap.rearrange("(c p) w -> p c w", p=128),
                )
                return t, [t[0:cn, ci * width:ci * width + width] for ci, (cs, cn) in enumerate(CI)]

            def proj_full(dst, w_tiles, x_tiles, m):
                """dst[m] [128, S] bf16 <- (w m-chunk).T @ x, one psum pass."""
                ps = psb.tile([128, S], F32, name=f"{P}pp{dst[m].name}", tag="big")
                for sh in range(2):
                    for ci in range(3):
                        nc.tensor.matmul(
                            ps[:, sh * 512:(sh + 1) * 512],
                            w_tiles[ci][:, m * 128:(m + 1) * 128],
                            x_tiles[ci][:, sh * 512:(sh + 1) * 512],
                            start=(ci == 0),
                            stop=(ci == 2),
                        )
                nc.vector.tensor_copy(dst[m][:], ps[:])

            def scores_exp(f, hp, tt):
                """Scores + exp for one (unit, t-tile) step; returns pt pair."""
                pts = [None, None]
                for par in range(2):
                    st = psb.tile([128, S], F32, name=f"{P}st{f}{hp}{tt}{par}", tag="big")
                    hl = par * DP
                    for sh in range(2):
                        nc.tensor.matmul(
                            st[:, sh * 512:(sh + 1) * 512],
                            kT8[hp][hl:hl + DP, tt * 128:(tt + 1) * 128],
                            qT8[f][hp][hl:hl + DP, sh * 512:(sh + 1) * 512],
                            start=True,
                            stop=True,
                        )
                    pt = ppt.tile([128, S], BF16, name=f"{P}pt{f}{hp}{tt}{par}", tag="pt")
                    emit_exp(pt, st, tt)
                    pts[par] = pt
                return pts

            def pv_step(hp, tt, pv, pts):
                for par in range(2):
                    h = hp * 2 + par
                    for si in range(8):
                        # psum pending-zero is bank-granular (2KB): one start
                        # marks the whole bank; each si's first write then
                        # zero-fills its own region
                        nc.tensor.matmul(
                            pv[par][:, si * DP:(si + 1) * DP],
                            pts[par][:, si * 128:(si + 1) * 128],
                            v8[tt][:, h * DP:(h + 1) * DP],
                            start=(tt == 0 and si == 0),
                            stop=(tt == 7),
                        )

            def finish_unit(f, hp, pv, pe_flip=False):
                """normalize (psum col 63 of each si-block is the softmax
                denominator) and flip [s, dp] -> [dp, s]. Mid-stream units
                flip on the (idle) DMA engines; the last unit flips on PE
                (is_transpose matmuls) to keep the tail off the serialized
                hwdge queue."""
                norm = pnorm.tile([128, S], BF16, name=f"{P}nm{f}{hp}", tag="nm")
                normv = norm[:].rearrange("p (si c) -> p si c", c=128)
                for par in range(2):
                    pvv = pv[par][:].rearrange("p (si c) -> p si c", c=DP)
                    rc = prc.tile([128, 8], F32, name=f"{P}rc{f}{hp}{par}", tag="rc")
                    nc.vector.reciprocal(rc[:], pvv[:, :, DP - 1])
                    nc.vector.tensor_mul(
                        normv[:, :, par * DP:(par + 1) * DP],
                        pvv[:],
                        rc[:].unsqueeze(-1).broadcast_to([128, 8, DP]),
                    )
                if pe_flip:
                    trp = psv.tile([128, S], BF16, name=f"{P}trp{f}{hp}", tag="pv")
                    for si in range(8):
                        nc.tensor.matmul(
                            trp[:, si * 128:(si + 1) * 128],
                            norm[:, si * 128:(si + 1) * 128],
                            ident_sb[:],
                            is_transpose=True,
                            start=True,
                            stop=True,
                        )
                    nc.vector.tensor_copy(outT[f][hp][:], trp[:])
                else:
                    for si in range(8):
                        nc.sync.dma_start_transpose(
                            outT[f][hp][:, si * 128:(si + 1) * 128],
                            norm[:, si * 128:(si + 1) * 128],
                        )

            def vproj(tt):
                ps = psb.tile([128, S], F32, name=f"{P}ppv{tt}", tag="big")
                for ci in range(3):
                    nc.tensor.matmul(
                        ps[:, 0:C],
                        x0_sb[ci][:, tt * 128:(tt + 1) * 128],
                        wv_sb[ci][:],
                        start=(ci == 0),
                        stop=(ci == 2),
                    )
                vv = v8[tt][:].rearrange("p (h c) -> p h c", c=DP)
                nc.vector.tensor_copy(
                    vv[:, :, 0:D],
                    ps[:, 0:C].rearrange("p (h c) -> p h c", c=D),
                )

            def oproj_mm(f, m, ps, sh, cps=range(4)):
                cos, cn = CO[m]
                for cp in cps:
                    nc.tensor.matmul(
                        ps[:, sh * 512:(sh + 1) * 512],
                        wo_sb[cp][:, cos:cos + cn],
                        outT[f][cp][:, sh * 512:(sh + 1) * 512],
                        start=(cp == 0),
                        stop=(cp == 3),
                    )

            def oproj_fin(f, m, ps, last=False):
                cos, cn = CO[m]
                y_sb = py.tile([cn, S], BF16, name=f"{P}y{f}{m}", tag=f"y{m}")
                # mid-stream y DMAs go via the (idle) gpsimd software DGE so
                # the SP hwdge queue stays free for the outT transposes; the
                # tail ones use the (by then free) fast hwdge path, split in
                # halves so the first DMA overlaps the second bias
                eng = nc.sync if last else nc.gpsimd
                for sh in range(2) if last else (slice(None),):
                    sl = (
                        slice(sh * 512, (sh + 1) * 512)
                        if isinstance(sh, int)
                        else sh
                    )
                    nc.scalar.activation(
                        y_sb[:, sl], ps[:, sl],
                        mybir.ActivationFunctionType.Identity, bias=bo_sb[m][:],
                    )
                    eng.dma_start(yt.ap()[f, cos:cos + cn, sl], y_sb[:, sl])

            def oproj(f, m):
                cos, cn = CO[m]
                ps = psb.tile([cn, S], F32, name=f"{P}oy{f}{m}", tag="big")
                for sh in range(2):
                    oproj_mm(f, m, ps, sh)
                oproj_fin(f, m, ps)

            # ---- input DMAs (hot-path order; x tiles chunked so the hot
            # ---- projections start as soon as each c_in chunk lands)
            whot_t = pconst.tile([128, 3 * 256], BF16, name=f"{P}whota", tag="whota")
            whot_ap3 = whot.ap().rearrange("(c p) w -> p c w", p=128)
            whot_tv = whot_t[:].rearrange("p (c w) -> p c w", w=256)
            whot_v = [
                whot_t[0:cn, ci * 256:ci * 256 + 256] for ci, (cs, cn) in enumerate(CI)
            ]
            whot_k = [t[:, 0:128] for t in whot_v]
            whot_q = [t[:, 128:256] for t in whot_v]

            def chunked_x_dma(nm, dram_ap):
                t = pconst.tile([128, 3 * S], BF16, name=f"{P}{nm}", tag=nm)
                for ci in range(3):
                    nc.sync.dma_start(
                        t[:, ci * S:(ci + 1) * S],
                        dram_ap[ci * 128:(ci + 1) * 128, :],
                    )
                return t, [t[0:cn, ci * S:ci * S + S] for ci, (cs, cn) in enumerate(CI)]

            nc.sync.dma_start(whot_tv[:, :, 0:128], whot_ap3[:, :, 0:128])
            _, x0_sb = chunked_x_dma("x0a", xt0.ap())
            nc.sync.dma_start(whot_tv[:, :, 128:256], whot_ap3[:, :, 128:256])
            xf_sb = [None, None]
            _, xf_sb[0] = chunked_x_dma("xfa0", xtf.ap()[0])

            proj_full(kT8, whot_k, x0_sb, 0)
            proj_full(qT8[0], whot_q, xf_sb[0], 0)

            _, wv_sb = one_dma_tile(pconst, "wva", wvp.ap(), C)
            wkq_t = pconst.tile([128, 3 * 2 * CP], BF16, name=f"{P}wkqa", tag="wkqa")
            wkq_ap3 = wkq.ap().rearrange("(c p) w -> p c w", p=128)
            wkq_tv = wkq_t[:].rearrange("p (c w) -> p c w", w=2 * CP)
            nc.sync.dma_start(wkq_tv[:, :, 0:CP], wkq_ap3[:, :, 0:CP])
            nc.sync.dma_start(wkq_tv[:, :, CP:2 * CP], wkq_ap3[:, :, CP:2 * CP])
            wkq_v = [
                wkq_t[0:cn, ci * 2 * CP:ci * 2 * CP + 2 * CP]
                for ci, (cs, cn) in enumerate(CI)
            ]
            wk_sb = [t[:, 0:CP] for t in wkq_v]
            wq_sb = [t[:, CP:2 * CP] for t in wkq_v]
            _, xf_sb[1] = chunked_x_dma("xfa1", xtf.ap()[1])
            wo_all = pconst.tile([128, 4 * C], BF16, name=f"{P}wo", tag="wo")
            wo_sb = [wo_all[:, cp * C:(cp + 1) * C] for cp in range(4)]
            nc.sync.dma_start(
                wo_all[:].rearrange("p (cp c) -> p cp c", c=C),
                wo.ap().rearrange("(cp p) c -> p cp c", p=128),
            )
            bo_all = pconst.tile([128, 3], F32, name=f"{P}bo", tag="bo")
            nc.sync.dma_start(bo_all[:], bo.ap().rearrange("(c p) -> p c", p=128))
            bo_sb = [bo_all[0:cn, m:m + 1] for m, (cs, cn) in enumerate(CO)]
            ident_sb = pconst.tile([128, 128], BF16, name=f"{P}ident", tag="ident")
            nc.sync.dma_start(ident_sb[:], ident.ap())

            def pg(dst, w_t, x_t, ms):
                return [
                    (lambda m=m: proj_full(dst, w_t, x_t, m)) for m in ms
                ]

            # ---- globally software-pipelined attention: PV lags scores/exp
            # ---- by one step ACROSS unit boundaries so PE never restarts
            UNITS = [(0, 0), (0, 1), (0, 2), (0, 3), (1, 0), (1, 1), (1, 2), (1, 3)]
            extras_by_unit = [
                pg(kT8, wk_sb, x0_sb, [1]) + pg(qT8[0], wq_sb, xf_sb[0], [1]),
                pg(kT8, wk_sb, x0_sb, [2]) + pg(qT8[0], wq_sb, xf_sb[0], [2]),
                pg(kT8, wk_sb, x0_sb, [3]) + pg(qT8[0], wq_sb, xf_sb[0], [3]),
                pg(qT8[1], wq_sb, xf_sb[1], [0, 1]),
                pg(qT8[1], wq_sb, xf_sb[1], [2, 3]),
                [lambda: oproj(0, 0), lambda: oproj(0, 1)],
                [lambda: oproj(0, 2)],
                [],
            ]
            # PV lags scores/exp by LAG steps so the PV group's exp dependency
            # is already satisfied at dispatch time (the PE sequencer is
            # in-order with a shallow wait queue - a blocked head instruction
            # stalls dispatch of everything behind it)
            NG = len(UNITS) * 8
            LAG = 2
            pts_hist: dict = {}
            pv_cur = None
            oy = None
            for g in range(NG + LAG):
                if g < NG:
                    u, tt = divmod(g, 8)
                    pts_hist[g] = scores_exp(*UNITS[u], tt)
                    if g < 8:
                        vproj(g)
                gp = g - LAG
                if gp >= 0:
                    up, ttp = divmod(gp, 8)
                    fp, hpp = UNITS[up]
                    if ttp == 0:
                        pv_cur = [
                            psv.tile([128, 512], F32, name=f"{P}pv{fp}{hpp}{par}", tag="pv")
                            for par in range(2)
                        ]
                    pv_step(hpp, ttp, pv_cur, pts_hist.pop(gp))
                    if ttp % 2 == 1 and extras_by_unit[up]:
                        extras_by_unit[up].pop(0)()
                    if ttp == 7:
                        last = up == len(UNITS) - 1
                        if last:
                            # pre-issue the frame-1 O-proj partials that only
                            # need outT[1][0..2]; they hide the last unit's
                            # normalize + flip latency
                            oy = [
                                psb.tile([CO[m][1], S], F32, name=f"{P}oy1{m}", tag="big")
                                for m in range(3)
                            ]
                            for m in range(3):
                                for sh in range(2):
                                    oproj_mm(1, m, oy[m], sh, cps=range(3))
                        finish_unit(fp, hpp, pv_cur, pe_flip=last)
            for m in range(3):
                for sh in range(2):
                    oproj_mm(1, m, oy[m], sh, cps=[3])
                oproj_fin(1, m, oy[m], last=True)

    nc.compile()
    return nc


def _get_nc(loop_n: int = 1):
    if loop_n not in _NC_CACHE:
        _NC_CACHE[loop_n] = _build(loop_n)
    return _NC_CACHE[loop_n]


def _pad_heads_cols(wT: np.ndarray) -> np.ndarray:
    """[C, C] (c_in, c_out) -> [C, CP] with each head's 40 cols at h*64."""
    out = np.zeros((C, CP), np.float32)
    out.reshape(C, H, DP)[:, :, :D] = wT.reshape(C, H, D)
    return out


def _prep_inputs(hidden_states, Wq, Wk, Wv, Wo, bo, video_length, k):
    hidden_states = np.asarray(hidden_states, dtype=np.float32)
    B = hidden_states.shape[0]
    assert hidden_states.shape == (B, S, C), hidden_states.shape
    assert B == NCORES * FPC, B
    kf = int(k)
    vl = int(video_length)
    b = B // vl
    assert b == 1, "kernel specialized for batch 1 (b*video_length == B)"

    xT = np.zeros((B, CPAD, S), np.float32)
    xT[:, :C, :] = hidden_states.transpose(0, 2, 1)
    xT = xT.astype(BF)
    wk_p = _pad_heads_cols(np.asarray(Wk, np.float32).T)
    wq_p = _pad_heads_cols(np.asarray(Wq, np.float32).T)
    wkq_p = np.zeros((CPAD, 2 * CP), np.float32)
    wkq_p[:C] = np.concatenate([wk_p, wq_p], axis=1)
    whot_p = np.zeros((CPAD, 256), np.float32)
    whot_p[:C] = np.concatenate([wk_p[:, 0:128], wq_p[:, 0:128]], axis=1)
    wv_p = np.zeros((CPAD, C), np.float32)
    wv_p[:C] = np.asarray(Wv, np.float32).T
    # WoT padded rows: row h*64+j = Wo[:, h*40+j]; pad rows (incl. the ones/
    # denominator row 63) are zero
    wo_p = np.zeros((CP, C), np.float32)
    wo_p.reshape(H, DP, C)[:, :D, :] = np.asarray(Wo, np.float32).T.reshape(H, D, C)
    bo_f = np.zeros(CPAD, np.float32)
    bo_f[:C] = np.asarray(bo, np.float32)

    xt0 = np.ascontiguousarray(xT[kf])
    wkq_b = wkq_p.astype(BF)
    whot_b = whot_p.astype(BF)
    wv_b = wv_p.astype(BF)
    wo_b = wo_p.astype(BF)
    ident_b = np.eye(128, dtype=np.float32).astype(BF)
    in_maps = []
    for c in range(NCORES):
        in_maps.append(
            {
                "xt0": xt0,
                "xtf": np.ascontiguousarray(xT[c * FPC:(c + 1) * FPC]),
                "wkq": wkq_b,
                "whot": whot_b,
                "wvp": wv_b,
                "wo": wo_b,
                "bo": bo_f,
                "ident": ident_b,
            }
        )
    return in_maps


def _run(inputs: dict, loop_n: int = 1):
    global LAST_RESULTS
    nc = _get_nc(loop_n)
    in_maps = _prep_inputs(**inputs)
    last_exc = None
    for _attempt in range(3):
        try:
            res = bass_utils.run_bass_kernel_spmd(nc, in_maps, core_ids=list(range(NCORES)))
            break
        except Exception as e:  # transient NRT/axon device hiccups
            last_exc = e
            import time as _time
            _time.sleep(2.0)
    else:
        raise last_exc
    LAST_RESULTS = res
    B = NCORES * FPC
    y = np.empty((B, S, C), np.float32)
    for c in range(NCORES):
        y[c * FPC:(c + 1) * FPC] = (
            res.results[c]["yt"].astype(np.float32).transpose(0, 2, 1)
        )
    return y


def kernel(hidden_states, Wq, Wk, Wv, Wo, bo, video_length, k):
    return _run(
        dict(
            hidden_states=hidden_states,
            Wq=Wq,
            Wk=Wk,
            Wv=Wv,
            Wo=Wo,
            bo=bo,
            video_length=video_length,
            k=k,
        )
    )


# revision 34
# speedup vs baseline: 1.0730x; 1.0033x over previous
"""Trainium2 Bass kernel for nn_BasicTransformerBlock (key-frame cross attention).

Reference computation (B=16 frames, S=1024, C=320, H=8 heads, D=40):
    q = x @ Wq.T ; k = x @ Wk.T ; v = x @ Wv.T
    k, v are taken from frame `kf` only and shared by every frame
    out = softmax(q k^T / sqrt(D)) v     (per frame, per head)
    y = out @ Wo.T + bo

Sharding: data-parallel over frames - 8 cores x 2 frames each. Every core
redundantly computes K/V from the key frame (cheap) so no collectives are
needed; outputs just concatenate.

Per-core design (all matmuls bf16 at full PE rate; psum accumulates f32):
  - Projections run "transposed": xT [C,S] tiles feed qT/kT [c_out_pad, S]
    (heads zero-padded D=40 -> 64 at 64-aligned partition offsets). V is
    projected as [t, 320] and repacked per head into v8 [t, {40 V | 23 zero
    | 1 ones}] bf16 blocks.
  - Scores st [t, s] = kT-slice.T @ qT (per head, per 128-row t-tile).
  - exp(score*scale) is SPLIT across two engines: the Scalar engine's Exp
    activation and a custom fused-DVE op EXPQ_ANT that evaluates
    ((c3 x + c2) x + c1) x + 1, squared twice (= exp to 4.6e-4 rel) in one
    8-stage DVE pass. Both write bf16 pt tiles. ~60/40 split balances
    Scalar vs Vector busy time.
  - PV runs TRANSPOSED: out[s-tile, 64] = pt-slice.T @ v8-block with pt as
    the stationary operand, so the matmul free dim is 64 (vs 1024 in the
    classic orientation) - PV drops from 54.6us to 27.3us of PE time. The
    ones column of v8 makes psum col 63 the softmax denominator per s-row,
    so normalization is one strided reciprocal [128,8] + one broadcast
    tensor_mul per head (per-partition scalars - no partition broadcast
    tricks needed).
  - The normalized [s, dp] tiles are flipped back to [dp, s] for the
    output projection by DMA-engine transposes (dma_start_transpose,
    [128,128] bf16 tiles) - the DMA engines are otherwise ~85% idle.
  - O-projection: y[c-chunk, s] = wo-chunk.T @ outT (+bias via Scalar
    Identity), DMAed out per chunk; host un-transposes.
  - psum: "big" ring 3x[128,1024] (scores double-buffered + proj/O-proj
    time-share, in allocation order - PE is strictly in-order) + "pv" ring
    2x[128,512]. All 8 banks.
"""

import os
import sys

import numpy as np

try:
    import concourse  # noqa: F401
except ImportError:  # pragma: no cover
    for _p in ("/opt/trn_rl_repo", os.path.dirname(os.path.abspath(__file__))):
        if os.path.isdir(os.path.join(_p, "concourse")):
            sys.path.insert(0, _p)
            break

import ml_dtypes  # noqa: E402

import concourse.dve_ops as dve_ops  # noqa: E402
import concourse.dve_spec as dve_spec  # noqa: E402
import concourse.mybir as mybir  # noqa: E402
import concourse.tile as tile  # noqa: E402
from concourse import bacc  # noqa: E402
from concourse import bass_utils  # noqa: E402
from concourse.dve_spec import C0, C1, C2, One, Spec, Src0, sq  # noqa: E402
from concourse.dve_uop import DveOpSpec  # noqa: E402

F32 = mybir.dt.float32
BF16 = mybir.dt.bfloat16
BF = ml_dtypes.bfloat16

S = 1024          # sequence length per frame
C = 320           # channels
H = 8             # heads
D = 40            # head dim
DP = 64           # padded head dim
CP = H * DP       # 512, padded channels
NCORES = 8
FPC = 2           # frames per core
SCALE = float(D) ** -0.5
CPAD = 384        # c_in padded to 3*128

CI = [(0, 128), (128, 128), (256, 64)]    # c_in chunks of 320
CO = [(0, 128), (128, 128), (256, 64)]    # c_out chunks of 320

# Of the 128 exp tiles per core, how many go to the Scalar engine (rest on
# the Vector engine's custom EXPQ op). Balances ACT vs DVE busy time.
ACT_TILES = 80

_NC_CACHE: dict = {}
LAST_RESULTS = None  # set by _run for test harness introspection


# --- custom DVE op: exp via deg-3 poly squared twice ------------------------

def _fit_exp_coefs(r=8.8, scale=SCALE):
    """exp(scale*x) ~= (((c3 x + c2) x + c1) x + 1)^4-ish: inner fits
    exp(scale*x/4) with relative-error weighting; max rel err ~4.6e-4 on
    |x| <= r (raw scores here are within +-8.4)."""
    k = np.arange(4000)
    xs = r * np.cos(np.pi * (k + 0.5) / 4000)
    w = 1.0 / np.exp(scale * xs / 4)
    A = np.stack([xs**p for p in range(1, 4)], 1)
    y = np.exp(scale * xs / 4) - 1.0
    c, *_ = np.linalg.lstsq(A * w[:, None], y * w, rcond=None)
    return c  # [c1, c2, c3]


EXP_COEF = _fit_exp_coefs()


def _register_expq():
    name = "EXPQ_ANT"
    for op in dve_ops.OPS:
        if op.name == name:
            return op
    x = Src0
    inner = ((C0 * x + C1) * x + C2) * x + One
    body = sq(sq(inner))

    def ref(in0, in1, s0, s1, imm2):
        xx = in0.astype(np.float32)
        p = ((s0 * xx + s1) * xx + imm2) * xx + 1.0
        return (p * p) ** 2

    spec = Spec(body=body, reference=ref)
    opcode = max(dve_ops._SUB_OPCODE_FOR_NAME.values()) + 1
    assert opcode < 0x20
    dve_ops._SUB_OPCODE_FOR_NAME[name] = opcode
    shas = {}
    for ver in ("v3", "v4"):
        s = DveOpSpec(
            name=name,
            opcode=opcode,
            uops=dve_spec.lower(spec, ver=ver),
            rd1_en=dve_spec._has_src1(spec),
        )
        shas[ver] = s.sha(ver)
    op = dve_ops.DveOp(name, spec, subdim=False, uops_sha=shas)
    dve_ops.OPS.append(op)
    dve_ops.CUSTOM_DVE_SPECS[name] = spec
    return op


EXPQ = _register_expq()


def _build(loop_n: int = 1):
    nc = bacc.Bacc("TRN2", target_bir_lowering=False, debug=False)

    xt0 = nc.dram_tensor("xt0", [CPAD, S], BF16, kind="ExternalInput")
    xtf = nc.dram_tensor("xtf", [FPC, CPAD, S], BF16, kind="ExternalInput")
    wkq = nc.dram_tensor("wkq", [CPAD, 2 * CP], BF16, kind="ExternalInput")
    whot = nc.dram_tensor("whot", [CPAD, 256], BF16, kind="ExternalInput")
    wvp = nc.dram_tensor("wvp", [CPAD, C], BF16, kind="ExternalInput")
    wo = nc.dram_tensor("wo", [CP, C], BF16, kind="ExternalInput")
    bo = nc.dram_tensor("bo", [CPAD], F32, kind="ExternalInput")
    ident = nc.dram_tensor("ident", [128, 128], BF16, kind="ExternalInput")
    yt = nc.dram_tensor("yt", [FPC, C, S], BF16, kind="ExternalOutput")

    c3f, c2f, c1f = float(EXP_COEF[2]), float(EXP_COEF[1]), float(EXP_COEF[0])

    with tile.TileContext(nc) as tc:
        with (
            tc.tile_pool(name="pconst", bufs=1) as pconst,
            tc.tile_pool(name="pqk", bufs=1) as pqk,
            tc.tile_pool(name="pvs", bufs=1) as pvs,
            tc.tile_pool(name="ppt", bufs=6) as ppt,
            tc.tile_pool(name="pnorm", bufs=3) as pnorm,
            tc.tile_pool(name="pot", bufs=1) as pot,
            tc.tile_pool(name="prc", bufs=4) as prc,
            tc.tile_pool(name="py", bufs=1) as py,
            tc.tile_pool(name="psb", bufs=3, space="PSUM") as psb,
            tc.tile_pool(name="psv", bufs=2, space="PSUM") as psv,
        ):
          for it in range(loop_n):
            P = f"{it}_"

            kT8 = [pqk.tile([128, S], BF16, name=f"{P}kT{m}", tag=f"kT{m}") for m in range(4)]
            qT8 = [
                [pqk.tile([128, S], BF16, name=f"{P}qT{f}_{m}", tag=f"qT{f}_{m}") for m in range(4)]
                for f in range(FPC)
            ]
            v8 = [pvs.tile([128, H * DP], BF16, name=f"{P}v{tt}", tag=f"v{tt}") for tt in range(8)]
            outT = [
                [pot.tile([128, S], BF16, name=f"{P}oT{f}_{hp}", tag=f"oT{f}_{hp}") for hp in range(4)]
                for f in range(FPC)
            ]

            # static v8 init: zero pad cols + ones col per head
            for tt in range(8):
                vv = v8[tt][:].rearrange("p (h c) -> p h c", c=DP)
                nc.gpsimd.memset(vv[:, :, D:DP], 0.0)
                nc.gpsimd.memset(vv[:, :, DP - 1:DP], 1.0)

            exp_ctr = [0]

            def emit_exp(pt_t, st_t, tt):
                i = exp_ctr[0]
                exp_ctr[0] += 1
                on_act = ((i + 1) * ACT_TILES) // 128 > (i * ACT_TILES) // 128
                if on_act:
                    nc.scalar.activation(
                        pt_t[:], st_t[:], mybir.ActivationFunctionType.Exp, scale=SCALE
                    )
                else:
                    nc.vector._custom_dve(
                        EXPQ, out=pt_t[:], in0=st_t[:], s0=c3f, s1=c2f, imm2=c1f
                    )

            def one_dma_tile(pool, nm, dram_ap, width):
                """[128, 3*width] bf16 tile <- [384, width] dram via one DMA."""
                t = pool.tile([128, 3 * width], BF16, name=f"{P}{nm}", tag=nm)
                nc.sync.dma_start(
                    t[:].rearrange("p (c w) -> p c w", w=width),
                    dram_ap.rearrange("(c p) w -> p c w", p=128),
                )
                return t, [t[0:cn, ci * width:ci * width + width] for ci, (cs, cn) in enumerate(CI)]

            def proj_full(dst, w_tiles, x_tiles, m):
                """dst[m] [128, S] bf16 <- (w m-chunk).T @ x, one psum pass."""
                ps = psb.tile([128, S], F32, name=f"{P}pp{dst[m].name}", tag="big")
                for sh in range(2):
                    for ci in range(3):
                        nc.tensor.matmul(
                            ps[:, sh * 512:(sh + 1) * 512],
                            w_tiles[ci][:, m * 128:(m + 1) * 128],
                            x_tiles[ci][:, sh * 512:(sh + 1) * 512],
                            start=(ci == 0),
                            stop=(ci == 2),
                        )
                nc.vector.tensor_copy(dst[m][:], ps[:])

            def scores_exp(f, hp, tt):
                """Scores + exp for one (unit, t-tile) step; returns pt pair."""
                pts = [None, None]
                for par in range(2):
                    st = psb.tile([128, S], F32, name=f"{P}st{f}{hp}{tt}{par}", tag="big")
                    hl = par * DP
                    for sh in range(2):
                        nc.tensor.matmul(
                            st[:, sh * 512:(sh + 1) * 512],
                            kT8[hp][hl:hl + DP, tt * 128:(tt + 1) * 128],
                            qT8[f][hp][hl:hl + DP, sh * 512:(sh + 1) * 512],
                            start=True,
                            stop=True,
                        )
                    pt = ppt.tile([128, S], BF16, name=f"{P}pt{f}{hp}{tt}{par}", tag="pt")
                    emit_exp(pt, st, tt)
                    pts[par] = pt
                return pts

            def pv_step(hp, tt, pv, pts):
                for par in range(2):
                    h = hp * 2 + par
                    for si in range(8):
                        # psum pending-zero is bank-granular (2KB): one start
                        # marks the whole bank; each si's first write then
                        # zero-fills its own region
                        nc.tensor.matmul(
                            pv[par][:, si * DP:(si + 1) * DP],
                            pts[par][:, si * 128:(si + 1) * 128],
                            v8[tt][:, h * DP:(h + 1) * DP],
                            start=(tt == 0 and si == 0),
                            stop=(tt == 7),
                        )

            def finish_unit(f, hp, pv, pe_flip=False):
                """normalize (psum col 63 of each si-block is the softmax
                denominator) and flip [s, dp] -> [dp, s]. Mid-stream units
                flip on the (idle) DMA engines; the last unit flips on PE
                (is_transpose matmuls) to keep the tail off the serialized
                hwdge queue."""
                norm = pnorm.tile([128, S], BF16, name=f"{P}nm{f}{hp}", tag="nm")
                normv = norm[:].rearrange("p (si c) -> p si c", c=128)
                for par in range(2):
                    pvv = pv[par][:].rearrange("p (si c) -> p si c", c=DP)
                    rc = prc.tile([128, 8], F32, name=f"{P}rc{f}{hp}{par}", tag="rc")
                    nc.vector.reciprocal(rc[:], pvv[:, :, DP - 1])
                    nc.vector.tensor_mul(
                        normv[:, :, par * DP:(par + 1) * DP],
                        pvv[:],
                        rc[:].unsqueeze(-1).broadcast_to([128, 8, DP]),
                    )
                if pe_flip:
                    trp = psv.tile([128, S], BF16, name=f"{P}trp{f}{hp}", tag="pv")
                    for si in range(8):
                        nc.tensor.matmul(
                            trp[:, si * 128:(si + 1) * 128],
                            norm[:, si * 128:(si + 1) * 128],
                            ident_sb[:],
                            is_transpose=True,
                            start=True,
                            stop=True,
                        )
                    nc.vector.tensor_copy(outT[f][hp][:], trp[:])
                else:
                    for si in range(8):
                        nc.sync.dma_start_transpose(
                            outT[f][hp][:, si * 128:(si + 1) * 128],
                            norm[:, si * 128:(si + 1) * 128],
                        )

            def vproj(tt):
                ps = psb.tile([128, S], F32, name=f"{P}ppv{tt}", tag="big")
                for ci in range(3):
                    nc.tensor.matmul(
                        ps[:, 0:C],
                        x0_sb[ci][:, tt * 128:(tt + 1) * 128],
                        wv_sb[ci][:],
                        start=(ci == 0),
                        stop=(ci == 2),
                    )
                vv = v8[tt][:].rearrange("p (h c) -> p h c", c=DP)
                nc.vector.tensor_copy(
                    vv[:, :, 0:D],
                    ps[:, 0:C].rearrange("p (h c) -> p h c", c=D),
                )

            def oproj_mm(f, m, ps, sh, cps=range(4)):
                cos, cn = CO[m]
                for cp in cps:
                    nc.tensor.matmul(
                        ps[:, sh * 512:(sh + 1) * 512],
                        wo_sb[cp][:, cos:cos + cn],
                        outT[f][cp][:, sh * 512:(sh + 1) * 512],
                        start=(cp == 0),
                        stop=(cp == 3),
                    )

            def oproj_fin(f, m, ps, last=False):
                cos, cn = CO[m]
                y_sb = py.tile([cn, S], BF16, name=f"{P}y{f}{m}", tag=f"y{m}")
                # mid-stream y DMAs go via the (idle) gpsimd software DGE so
                # the SP hwdge queue stays free for the outT transposes; the
                # tail ones use the (by then free) fast hwdge path, split in
                # halves so the first DMA overlaps the second bias
                eng = nc.sync if last else nc.gpsimd
                for sh in range(2) if last else (slice(None),):
                    sl = (
                        slice(sh * 512, (sh + 1) * 512)
                        if isinstance(sh, int)
                        else sh
                    )
                    nc.scalar.activation(
                        y_sb[:, sl], ps[:, sl],
                        mybir.ActivationFunctionType.Identity, bias=bo_sb[m][:],
                    )
                    eng.dma_start(yt.ap()[f, cos:cos + cn, sl], y_sb[:, sl])

            def oproj(f, m):
                cos, cn = CO[m]
                ps = psb.tile([cn, S], F32, name=f"{P}oy{f}{m}", tag="big")
                for sh in range(2):
                    oproj_mm(f, m, ps, sh)
                oproj_fin(f, m, ps)

            # ---- input DMAs (hot-path order; x tiles chunked so the hot
            # ---- projections start as soon as each c_in chunk lands)
            whot_t = pconst.tile([128, 3 * 256], BF16, name=f"{P}whota", tag="whota")
            whot_ap3 = whot.ap().rearrange("(c p) w -> p c w", p=128)
            whot_tv = whot_t[:].rearrange("p (c w) -> p c w", w=256)
            whot_v = [
                whot_t[0:cn, ci * 256:ci * 256 + 256] for ci, (cs, cn) in enumerate(CI)
            ]
            whot_k = [t[:, 0:128] for t in whot_v]
            whot_q = [t[:, 128:256] for t in whot_v]

            def chunked_x_dma(nm, dram_ap):
                t = pconst.tile([128, 3 * S], BF16, name=f"{P}{nm}", tag=nm)
                for ci in range(3):
                    nc.sync.dma_start(
                        t[:, ci * S:(ci + 1) * S],
                        dram_ap[ci * 128:(ci + 1) * 128, :],
                    )
                return t, [t[0:cn, ci * S:ci * S + S] for ci, (cs, cn) in enumerate(CI)]

            nc.sync.dma_start(whot_tv[:, :, 0:128], whot_ap3[:, :, 0:128])
            _, x0_sb = chunked_x_dma("x0a", xt0.ap())
            nc.sync.dma_start(whot_tv[:, :, 128:256], whot_ap3[:, :, 128:256])
            xf_sb = [None, None]
            _, xf_sb[0] = chunked_x_dma("xfa0", xtf.ap()[0])

            proj_full(kT8, whot_k, x0_sb, 0)
            proj_full(qT8[0], whot_q, xf_sb[0], 0)

            _, wv_sb = one_dma_tile(pconst, "wva", wvp.ap(), C)
            wkq_t = pconst.tile([128, 3 * 2 * CP], BF16, name=f"{P}wkqa", tag="wkqa")
            wkq_ap3 = wkq.ap().rearrange("(c p) w -> p c w", p=128)
            wkq_tv = wkq_t[:].rearrange("p (c w) -> p c w", w=2 * CP)
            nc.sync.dma_start(wkq_tv[:, :, 0:CP], wkq_ap3[:, :, 0:CP])
            nc.sync.dma_start(wkq_tv[:, :, CP:2 * CP], wkq_ap3[:, :, CP:2 * CP])
            wkq_v = [
                wkq_t[0:cn, ci * 2 * CP:ci * 2 * CP + 2 * CP]
                for ci, (cs, cn) in enumerate(CI)
            ]
            wk_sb = [t[:, 0:CP] for t in wkq_v]
            wq_sb = [t[:, CP:2 * CP] for t in wkq_v]
            _, xf_sb[1] = chunked_x_dma("xfa1", xtf.ap()[1])
            wo_all = pconst.tile([128, 4 * C], BF16, name=f"{P}wo", tag="wo")
            wo_sb = [wo_all[:, cp * C:(cp + 1) * C] for cp in range(4)]
            nc.sync.dma_start(
                wo_all[:].rearrange("p (cp c) -> p cp c", c=C),
                wo.ap().rearrange("(cp p) c -> p cp c", p=128),
            )
            bo_all = pconst.tile([128, 3], F32, name=f"{P}bo", tag="bo")
            nc.sync.dma_start(bo_all[:], bo.ap().rearrange("(c p) -> p c", p=128))
            bo_sb = [bo_all[0:cn, m:m + 1] for m, (cs, cn) in enumerate(CO)]
            ident_sb = pconst.tile([128, 128], BF16, name=f"{P}ident", tag="ident")
            nc.sync.dma_start(ident_sb[:], ident.ap())

            vproj(0)

            def pg(dst, w_t, x_t, ms):
                return [
                    (lambda m=m: proj_full(dst, w_t, x_t, m)) for m in ms
                ]

            # ---- globally software-pipelined attention: PV lags scores/exp
            # ---- by one step ACROSS unit boundaries so PE never restarts
            UNITS = [(0, 0), (0, 1), (0, 2), (0, 3), (1, 0), (1, 1), (1, 2), (1, 3)]
            extras_by_unit = [
                pg(kT8, wk_sb, x0_sb, [1]) + pg(qT8[0], wq_sb, xf_sb[0], [1]),
                pg(kT8, wk_sb, x0_sb, [2]) + pg(qT8[0], wq_sb, xf_sb[0], [2]),
                pg(kT8, wk_sb, x0_sb, [3]) + pg(qT8[0], wq_sb, xf_sb[0], [3]),
                pg(qT8[1], wq_sb, xf_sb[1], [0, 1]),
                pg(qT8[1], wq_sb, xf_sb[1], [2, 3]),
                [lambda: oproj(0, 0), lambda: oproj(0, 1)],
                [lambda: oproj(0, 2)],
                [],
            ]
            # PV lags scores/exp by LAG steps so the PV group's exp dependency
            # is already satisfied at dispatch time (the PE sequencer is
            # in-order with a shallow wait queue - a blocked head instruction
            # stalls dispatch of everything behind it)
            NG = len(UNITS) * 8
            LAG = 2
            pts_hist: dict = {}
            pv_cur = None
            oy = None
            for g in range(NG + LAG):
                if g < NG:
                    u, tt = divmod(g, 8)
                    pts_hist[g] = scores_exp(*UNITS[u], tt)
                    if g < 7:
                        vproj(g + 1)
                gp = g - LAG
                if gp >= 0:
                    up, ttp = divmod(gp, 8)
                    fp, hpp = UNITS[up]
                    if ttp == 0:
                        pv_cur = [
                            psv.tile([128, 512], F32, name=f"{P}pv{fp}{hpp}{par}", tag="pv")
                            for par in range(2)
                        ]
                    pv_step(hpp, ttp, pv_cur, pts_hist.pop(gp))
                    if ttp % 2 == 1 and extras_by_unit[up]:
                        extras_by_unit[up].pop(0)()
                    if ttp == 7:
                        last = up == len(UNITS) - 1
                        if last:
                            # pre-issue the frame-1 O-proj partials that only
                            # need outT[1][0..2]; they hide the last unit's
                            # normalize + flip latency
                            oy = [
                                psb.tile([CO[m][1], S], F32, name=f"{P}oy1{m}", tag="big")
                                for m in range(3)
                            ]
                            for m in range(3):
                                for sh in range(2):
                                    oproj_mm(1, m, oy[m], sh, cps=range(3))
                        finish_unit(fp, hpp, pv_cur, pe_flip=last)
            for m in range(3):
                for sh in range(2):
                    oproj_mm(1, m, oy[m], sh, cps=[3])
                oproj_fin(1, m, oy[m], last=True)

    nc.compile()
    return nc


def _get_nc(loop_n: int = 1):
    if loop_n not in _NC_CACHE:
        _NC_CACHE[loop_n] = _build(loop_n)
    return _NC_CACHE[loop_n]


def _pad_heads_cols(wT: np.ndarray) -> np.ndarray:
    """[C, C] (c_in, c_out) -> [C, CP] with each head's 40 cols at h*64."""
    out = np.zeros((C, CP), np.float32)
    out.reshape(C, H, DP)[:, :, :D] = wT.reshape(C, H, D)
    return out


def _prep_inputs(hidden_states, Wq, Wk, Wv, Wo, bo, video_length, k):
    hidden_states = np.asarray(hidden_states, dtype=np.float32)
    B = hidden_states.shape[0]
    assert hidden_states.shape == (B, S, C), hidden_states.shape
    assert B == NCORES * FPC, B
    kf = int(k)
    vl = int(video_length)
    b = B // vl
    assert b == 1, "kernel specialized for batch 1 (b*video_length == B)"

    xT = np.zeros((B, CPAD, S), np.float32)
    xT[:, :C, :] = hidden_states.transpose(0, 2, 1)
    xT = xT.astype(BF)
    wk_p = _pad_heads_cols(np.asarray(Wk, np.float32).T)
    wq_p = _pad_heads_cols(np.asarray(Wq, np.float32).T)
    wkq_p = np.zeros((CPAD, 2 * CP), np.float32)
    wkq_p[:C] = np.concatenate([wk_p, wq_p], axis=1)
    whot_p = np.zeros((CPAD, 256), np.float32)
    whot_p[:C] = np.concatenate([wk_p[:, 0:128], wq_p[:, 0:128]], axis=1)
    wv_p = np.zeros((CPAD, C), np.float32)
    wv_p[:C] = np.asarray(Wv, np.float32).T
    # WoT padded rows: row h*64+j = Wo[:, h*40+j]; pad rows (incl. the ones/
    # denominator row 63) are zero
    wo_p = np.zeros((CP, C), np.float32)
    wo_p.reshape(H, DP, C)[:, :D, :] = np.asarray(Wo, np.float32).T.reshape(H, D, C)
    bo_f = np.zeros(CPAD, np.float32)
    bo_f[:C] = np.asarray(bo, np.float32)

    xt0 = np.ascontiguousarray(xT[kf])
    wkq_b = wkq_p.astype(BF)
    whot_b = whot_p.astype(BF)
    wv_b = wv_p.astype(BF)
    wo_b = wo_p.astype(BF)
    ident_b = np.eye(128, dtype=np.float32).astype(BF)
    in_maps = []
    for c in range(NCORES):
        in_maps.append(
            {
                "xt0": xt0,
                "xtf": np.ascontiguousarray(xT[c * FPC:(c + 1) * FPC]),
                "wkq": wkq_b,
                "whot": whot_b,
                "wvp": wv_b,
                "wo": wo_b,
                "bo": bo_f,
                "ident": ident_b,
            }
        )
    return in_maps


def _run(inputs: dict, loop_n: int = 1):
    global LAST_RESULTS
    nc = _get_nc(loop_n)
    in_maps = _prep_inputs(**inputs)
    last_exc = None
    for _attempt in range(3):
        try:
            res = bass_utils.run_bass_kernel_spmd(nc, in_maps, core_ids=list(range(NCORES)))
            break
        except Exception as e:  # transient NRT/axon device hiccups
            last_exc = e
            import time as _time
            _time.sleep(2.0)
    else:
        raise last_exc
    LAST_RESULTS = res
    B = NCORES * FPC
    y = np.empty((B, S, C), np.float32)
    for c in range(NCORES):
        y[c * FPC:(c + 1) * FPC] = (
            res.results[c]["yt"].astype(np.float32).transpose(0, 2, 1)
        )
    return y


def kernel(hidden_states, Wq, Wk, Wv, Wo, bo, video_length, k):
    return _run(
        dict(
            hidden_states=hidden_states,
            Wq=Wq,
            Wk=Wk,
            Wv=Wv,
            Wo=Wo,
            bo=bo,
            video_length=video_length,
            k=k,
        )
    )


# revision 36
# speedup vs baseline: 1.0844x; 1.0107x over previous
"""Trainium2 Bass kernel for nn_BasicTransformerBlock (key-frame cross attention).

Reference computation (B=16 frames, S=1024, C=320, H=8 heads, D=40):
    q = x @ Wq.T ; k = x @ Wk.T ; v = x @ Wv.T
    k, v are taken from frame `kf` only and shared by every frame
    out = softmax(q k^T / sqrt(D)) v     (per frame, per head)
    y = out @ Wo.T + bo

Sharding: data-parallel over frames - 8 cores x 2 frames each. Every core
redundantly computes K/V from the key frame (cheap) so no collectives are
needed; outputs just concatenate.

Per-core design (all matmuls bf16 at full PE rate; psum accumulates f32):
  - Projections run "transposed": xT [C,S] tiles feed qT/kT [c_out_pad, S]
    (heads zero-padded D=40 -> 64 at 64-aligned partition offsets). V is
    projected as [t, 320] and repacked per head into v8 [t, {40 V | 23 zero
    | 1 ones}] bf16 blocks.
  - Scores st [t, s] = kT-slice.T @ qT (per head, per 128-row t-tile).
  - exp(score*scale) is SPLIT across two engines: the Scalar engine's Exp
    activation and a custom fused-DVE op EXPQ_ANT that evaluates
    ((c3 x + c2) x + c1) x + 1, squared twice (= exp to 4.6e-4 rel) in one
    8-stage DVE pass. Both write bf16 pt tiles. ~60/40 split balances
    Scalar vs Vector busy time.
  - PV runs TRANSPOSED: out[s-tile, 64] = pt-slice.T @ v8-block with pt as
    the stationary operand, so the matmul free dim is 64 (vs 1024 in the
    classic orientation) - PV drops from 54.6us to 27.3us of PE time. The
    ones column of v8 makes psum col 63 the softmax denominator per s-row,
    so normalization is one strided reciprocal [128,8] + one broadcast
    tensor_mul per head (per-partition scalars - no partition broadcast
    tricks needed).
  - The normalized [s, dp] tiles are flipped back to [dp, s] for the
    output projection by DMA-engine transposes (dma_start_transpose,
    [128,128] bf16 tiles) - the DMA engines are otherwise ~85% idle.
  - O-projection: y[c-chunk, s] = wo-chunk.T @ outT (+bias via Scalar
    Identity), DMAed out per chunk; host un-transposes.
  - psum: "big" ring 3x[128,1024] (scores double-buffered + proj/O-proj
    time-share, in allocation order - PE is strictly in-order) + "pv" ring
    2x[128,512]. All 8 banks.
"""

import os
import sys

import numpy as np

try:
    import concourse  # noqa: F401
except ImportError:  # pragma: no cover
    for _p in ("/opt/trn_rl_repo", os.path.dirname(os.path.abspath(__file__))):
        if os.path.isdir(os.path.join(_p, "concourse")):
            sys.path.insert(0, _p)
            break

import ml_dtypes  # noqa: E402

import concourse.dve_ops as dve_ops  # noqa: E402
import concourse.dve_spec as dve_spec  # noqa: E402
import concourse.mybir as mybir  # noqa: E402
import concourse.tile as tile  # noqa: E402
from concourse import bacc  # noqa: E402
from concourse import bass_utils  # noqa: E402
from concourse.dve_spec import C0, C1, C2, One, Spec, Src0, sq  # noqa: E402
from concourse.dve_uop import DveOpSpec  # noqa: E402

F32 = mybir.dt.float32
BF16 = mybir.dt.bfloat16
BF = ml_dtypes.bfloat16

S = 1024          # sequence length per frame
C = 320           # channels
H = 8             # heads
D = 40            # head dim
DP = 64           # padded head dim
CP = H * DP       # 512, padded channels
NCORES = 8
FPC = 2           # frames per core
SCALE = float(D) ** -0.5
CPAD = 384        # c_in padded to 3*128

CI = [(0, 128), (128, 128), (256, 64)]    # c_in chunks of 320
CO = [(0, 128), (128, 128), (256, 64)]    # c_out chunks of 320

# Of the 128 exp tiles per core, how many go to the Scalar engine (rest on
# the Vector engine's custom EXPQ op). Balances ACT vs DVE busy time.
ACT_TILES = 80

_NC_CACHE: dict = {}
LAST_RESULTS = None  # set by _run for test harness introspection


# --- custom DVE op: exp via deg-3 poly squared twice ------------------------

def _fit_exp_coefs(r=8.8, scale=SCALE):
    """exp(scale*x) ~= (((c3 x + c2) x + c1) x + 1)^4-ish: inner fits
    exp(scale*x/4) with relative-error weighting; max rel err ~4.6e-4 on
    |x| <= r (raw scores here are within +-8.4)."""
    k = np.arange(4000)
    xs = r * np.cos(np.pi * (k + 0.5) / 4000)
    w = 1.0 / np.exp(scale * xs / 4)
    A = np.stack([xs**p for p in range(1, 4)], 1)
    y = np.exp(scale * xs / 4) - 1.0
    c, *_ = np.linalg.lstsq(A * w[:, None], y * w, rcond=None)
    return c  # [c1, c2, c3]


EXP_COEF = _fit_exp_coefs()


def _register_expq():
    name = "EXPQ_ANT"
    for op in dve_ops.OPS:
        if op.name == name:
            return op
    x = Src0
    inner = ((C0 * x + C1) * x + C2) * x + One
    body = sq(sq(inner))

    def ref(in0, in1, s0, s1, imm2):
        xx = in0.astype(np.float32)
        p = ((s0 * xx + s1) * xx + imm2) * xx + 1.0
        return (p * p) ** 2

    spec = Spec(body=body, reference=ref)
    opcode = max(dve_ops._SUB_OPCODE_FOR_NAME.values()) + 1
    assert opcode < 0x20
    dve_ops._SUB_OPCODE_FOR_NAME[name] = opcode
    shas = {}
    for ver in ("v3", "v4"):
        s = DveOpSpec(
            name=name,
            opcode=opcode,
            uops=dve_spec.lower(spec, ver=ver),
            rd1_en=dve_spec._has_src1(spec),
        )
        shas[ver] = s.sha(ver)
    op = dve_ops.DveOp(name, spec, subdim=False, uops_sha=shas)
    dve_ops.OPS.append(op)
    dve_ops.CUSTOM_DVE_SPECS[name] = spec
    return op


EXPQ = _register_expq()


def _build(loop_n: int = 1):
    nc = bacc.Bacc("TRN2", target_bir_lowering=False, debug=False)

    xt0 = nc.dram_tensor("xt0", [CPAD, S], BF16, kind="ExternalInput")
    xtf = nc.dram_tensor("xtf", [FPC, CPAD, S], BF16, kind="ExternalInput")
    wkq = nc.dram_tensor("wkq", [CPAD, 2 * CP], BF16, kind="ExternalInput")
    whot = nc.dram_tensor("whot", [CPAD, 256], BF16, kind="ExternalInput")
    wvp = nc.dram_tensor("wvp", [CPAD, C], BF16, kind="ExternalInput")
    wo = nc.dram_tensor("wo", [CP, C], BF16, kind="ExternalInput")
    bo = nc.dram_tensor("bo", [CPAD], F32, kind="ExternalInput")
    ident = nc.dram_tensor("ident", [128, 128], BF16, kind="ExternalInput")
    yt = nc.dram_tensor("yt", [FPC, C, S], BF16, kind="ExternalOutput")

    c3f, c2f, c1f = float(EXP_COEF[2]), float(EXP_COEF[1]), float(EXP_COEF[0])

    with tile.TileContext(nc) as tc:
        with (
            tc.tile_pool(name="pconst", bufs=1) as pconst,
            tc.tile_pool(name="pqk", bufs=1) as pqk,
            tc.tile_pool(name="pvs", bufs=1) as pvs,
            tc.tile_pool(name="ppt", bufs=6) as ppt,
            tc.tile_pool(name="pnorm", bufs=3) as pnorm,
            tc.tile_pool(name="pot", bufs=1) as pot,
            tc.tile_pool(name="prc", bufs=4) as prc,
            tc.tile_pool(name="py", bufs=1) as py,
            tc.tile_pool(name="psb", bufs=3, space="PSUM") as psb,
            tc.tile_pool(name="psv", bufs=2, space="PSUM") as psv,
        ):
          for it in range(loop_n):
            P = f"{it}_"

            kT8 = [pqk.tile([128, S], BF16, name=f"{P}kT{m}", tag=f"kT{m}") for m in range(4)]
            qT8 = [
                [pqk.tile([128, S], BF16, name=f"{P}qT{f}_{m}", tag=f"qT{f}_{m}") for m in range(4)]
                for f in range(FPC)
            ]
            v8 = [pvs.tile([128, H * DP], BF16, name=f"{P}v{tt}", tag=f"v{tt}") for tt in range(8)]
            outT = [
                [pot.tile([128, S], BF16, name=f"{P}oT{f}_{hp}", tag=f"oT{f}_{hp}") for hp in range(4)]
                for f in range(FPC)
            ]

            # static v8 init: zero pad cols + ones col per head
            for tt in range(8):
                vv = v8[tt][:].rearrange("p (h c) -> p h c", c=DP)
                nc.gpsimd.memset(vv[:, :, D:DP], 0.0)
                nc.gpsimd.memset(vv[:, :, DP - 1:DP], 1.0)

            exp_ctr = [0]

            def emit_exp(pt_t, st_t, tt):
                i = exp_ctr[0]
                exp_ctr[0] += 1
                on_act = ((i + 1) * ACT_TILES) // 128 > (i * ACT_TILES) // 128
                if on_act:
                    nc.scalar.activation(
                        pt_t[:], st_t[:], mybir.ActivationFunctionType.Exp, scale=SCALE
                    )
                else:
                    nc.vector._custom_dve(
                        EXPQ, out=pt_t[:], in0=st_t[:], s0=c3f, s1=c2f, imm2=c1f
                    )

            def one_dma_tile(pool, nm, dram_ap, width):
                """[128, 3*width] bf16 tile <- [384, width] dram via one DMA."""
                t = pool.tile([128, 3 * width], BF16, name=f"{P}{nm}", tag=nm)
                nc.sync.dma_start(
                    t[:].rearrange("p (c w) -> p c w", w=width),
                    dram_ap.rearrange("(c p) w -> p c w", p=128),
                )
                return t, [t[0:cn, ci * width:ci * width + width] for ci, (cs, cn) in enumerate(CI)]

            def proj_full(dst, w_tiles, x_tiles, m):
                """dst[m] [128, S] bf16 <- (w m-chunk).T @ x, one psum pass."""
                ps = psb.tile([128, S], F32, name=f"{P}pp{dst[m].name}", tag="big")
                for sh in range(2):
                    for ci in range(3):
                        nc.tensor.matmul(
                            ps[:, sh * 512:(sh + 1) * 512],
                            w_tiles[ci][:, m * 128:(m + 1) * 128],
                            x_tiles[ci][:, sh * 512:(sh + 1) * 512],
                            start=(ci == 0),
                            stop=(ci == 2),
                        )
                nc.vector.tensor_copy(dst[m][:], ps[:])

            def scores_exp(f, hp, tt):
                """Scores + exp for one (unit, t-tile) step; returns pt pair."""
                pts = [None, None]
                for par in range(2):
                    st = psb.tile([128, S], F32, name=f"{P}st{f}{hp}{tt}{par}", tag="big")
                    hl = par * DP
                    for sh in range(2):
                        nc.tensor.matmul(
                            st[:, sh * 512:(sh + 1) * 512],
                            kT8[hp][hl:hl + DP, tt * 128:(tt + 1) * 128],
                            qT8[f][hp][hl:hl + DP, sh * 512:(sh + 1) * 512],
                            start=True,
                            stop=True,
                        )
                    pt = ppt.tile([128, S], BF16, name=f"{P}pt{f}{hp}{tt}{par}", tag="pt")
                    emit_exp(pt, st, tt)
                    pts[par] = pt
                return pts

            def pv_step(hp, tt, pv, pts):
                for par in range(2):
                    h = hp * 2 + par
                    for si in range(8):
                        # psum pending-zero is bank-granular (2KB): one start
                        # marks the whole bank; each si's first write then
                        # zero-fills its own region
                        nc.tensor.matmul(
                            pv[par][:, si * DP:(si + 1) * DP],
                            pts[par][:, si * 128:(si + 1) * 128],
                            v8[tt][:, h * DP:(h + 1) * DP],
                            start=(tt == 0 and si == 0),
                            stop=(tt == 7),
                        )

            def finish_unit(f, hp, pv, pe_flip=False):
                """normalize (psum col 63 of each si-block is the softmax
                denominator) and flip [s, dp] -> [dp, s]. Mid-stream units
                flip on the (idle) DMA engines; the last unit flips on PE
                (is_transpose matmuls) to keep the tail off the serialized
                hwdge queue."""
                norm = pnorm.tile([128, S], BF16, name=f"{P}nm{f}{hp}", tag="nm")
                normv = norm[:].rearrange("p (si c) -> p si c", c=128)
                for par in range(2):
                    pvv = pv[par][:].rearrange("p (si c) -> p si c", c=DP)
                    rc = prc.tile([128, 8], F32, name=f"{P}rc{f}{hp}{par}", tag="rc")
                    nc.vector.reciprocal(rc[:], pvv[:, :, DP - 1])
                    nc.vector.tensor_mul(
                        normv[:, :, par * DP:(par + 1) * DP],
                        pvv[:],
                        rc[:].unsqueeze(-1).broadcast_to([128, 8, DP]),
                    )
                if pe_flip:
                    trp = psv.tile([128, S], BF16, name=f"{P}trp{f}{hp}", tag="pv")
                    for si in range(8):
                        nc.tensor.matmul(
                            trp[:, si * 128:(si + 1) * 128],
                            norm[:, si * 128:(si + 1) * 128],
                            ident_sb[:],
                            is_transpose=True,
                            start=True,
                            stop=True,
                        )
                    nc.vector.tensor_copy(outT[f][hp][:], trp[:])
                else:
                    for si in range(8):
                        nc.sync.dma_start_transpose(
                            outT[f][hp][:, si * 128:(si + 1) * 128],
                            norm[:, si * 128:(si + 1) * 128],
                        )

            def vproj(tt):
                ps = psb.tile([128, S], F32, name=f"{P}ppv{tt}", tag="big")
                for ci in range(3):
                    nc.tensor.matmul(
                        ps[:, 0:C],
                        x0_sb[ci][:, tt * 128:(tt + 1) * 128],
                        wv_sb[ci][:],
                        start=(ci == 0),
                        stop=(ci == 2),
                    )
                vv = v8[tt][:].rearrange("p (h c) -> p h c", c=DP)
                nc.vector.tensor_copy(
                    vv[:, :, 0:D],
                    ps[:, 0:C].rearrange("p (h c) -> p h c", c=D),
                )

            def oproj_mm(f, m, ps, sh, cps=range(4)):
                cos, cn = CO[m]
                for cp in cps:
                    nc.tensor.matmul(
                        ps[:, sh * 512:(sh + 1) * 512],
                        wo_sb[cp][:, cos:cos + cn],
                        outT[f][cp][:, sh * 512:(sh + 1) * 512],
                        start=(cp == 0),
                        stop=(cp == 3),
                    )

            def oproj_fin(f, m, ps, last=False):
                cos, cn = CO[m]
                y_sb = py.tile([cn, S], BF16, name=f"{P}y{f}{m}", tag=f"y{m}")
                # mid-stream y DMAs go via the (idle) gpsimd software DGE so
                # the SP hwdge queue stays free for the outT transposes; the
                # tail ones use the (by then free) fast hwdge path, split in
                # halves so the first DMA overlaps the second bias
                eng = nc.sync if last else nc.gpsimd
                nc.scalar.activation(
                    y_sb[:], ps[:],
                    mybir.ActivationFunctionType.Identity, bias=bo_sb[m][:],
                )
                eng.dma_start(yt.ap()[f, cos:cos + cn, :], y_sb[:])

            def oproj(f, m):
                cos, cn = CO[m]
                ps = psb.tile([cn, S], F32, name=f"{P}oy{f}{m}", tag="big")
                for sh in range(2):
                    oproj_mm(f, m, ps, sh)
                oproj_fin(f, m, ps)

            # ---- input DMAs (hot-path order; x tiles chunked so the hot
            # ---- projections start as soon as each c_in chunk lands)
            whot_t = pconst.tile([128, 3 * 256], BF16, name=f"{P}whota", tag="whota")
            whot_ap3 = whot.ap().rearrange("(c p) w -> p c w", p=128)
            whot_tv = whot_t[:].rearrange("p (c w) -> p c w", w=256)
            whot_v = [
                whot_t[0:cn, ci * 256:ci * 256 + 256] for ci, (cs, cn) in enumerate(CI)
            ]
            whot_k = [t[:, 0:128] for t in whot_v]
            whot_q = [t[:, 128:256] for t in whot_v]

            def chunked_x_dma(nm, dram_ap):
                t = pconst.tile([128, 3 * S], BF16, name=f"{P}{nm}", tag=nm)
                for ci in range(3):
                    nc.sync.dma_start(
                        t[:, ci * S:(ci + 1) * S],
                        dram_ap[ci * 128:(ci + 1) * 128, :],
                    )
                return t, [t[0:cn, ci * S:ci * S + S] for ci, (cs, cn) in enumerate(CI)]

            nc.sync.dma_start(whot_tv[:, :, 0:128], whot_ap3[:, :, 0:128])
            _, x0_sb = chunked_x_dma("x0a", xt0.ap())
            nc.sync.dma_start(whot_tv[:, :, 128:256], whot_ap3[:, :, 128:256])
            xf_sb = [None, None]
            _, xf_sb[0] = chunked_x_dma("xfa0", xtf.ap()[0])

            proj_full(kT8, whot_k, x0_sb, 0)
            proj_full(qT8[0], whot_q, xf_sb[0], 0)

            _, wv_sb = one_dma_tile(pconst, "wva", wvp.ap(), C)
            wkq_t = pconst.tile([128, 3 * 2 * CP], BF16, name=f"{P}wkqa", tag="wkqa")
            wkq_ap3 = wkq.ap().rearrange("(c p) w -> p c w", p=128)
            wkq_tv = wkq_t[:].rearrange("p (c w) -> p c w", w=2 * CP)
            nc.sync.dma_start(wkq_tv[:, :, 0:CP], wkq_ap3[:, :, 0:CP])
            nc.sync.dma_start(wkq_tv[:, :, CP:2 * CP], wkq_ap3[:, :, CP:2 * CP])
            wkq_v = [
                wkq_t[0:cn, ci * 2 * CP:ci * 2 * CP + 2 * CP]
                for ci, (cs, cn) in enumerate(CI)
            ]
            wk_sb = [t[:, 0:CP] for t in wkq_v]
            wq_sb = [t[:, CP:2 * CP] for t in wkq_v]
            _, xf_sb[1] = chunked_x_dma("xfa1", xtf.ap()[1])
            wo_all = pconst.tile([128, 4 * C], BF16, name=f"{P}wo", tag="wo")
            wo_sb = [wo_all[:, cp * C:(cp + 1) * C] for cp in range(4)]
            nc.sync.dma_start(
                wo_all[:].rearrange("p (cp c) -> p cp c", c=C),
                wo.ap().rearrange("(cp p) c -> p cp c", p=128),
            )
            bo_all = pconst.tile([128, 3], F32, name=f"{P}bo", tag="bo")
            nc.sync.dma_start(bo_all[:], bo.ap().rearrange("(c p) -> p c", p=128))
            bo_sb = [bo_all[0:cn, m:m + 1] for m, (cs, cn) in enumerate(CO)]
            ident_sb = pconst.tile([128, 128], BF16, name=f"{P}ident", tag="ident")
            nc.sync.dma_start(ident_sb[:], ident.ap())

            vproj(0)

            def pg(dst, w_t, x_t, ms):
                return [
                    (lambda m=m: proj_full(dst, w_t, x_t, m)) for m in ms
                ]

            # ---- globally software-pipelined attention: PV lags scores/exp
            # ---- by one step ACROSS unit boundaries so PE never restarts
            UNITS = [(0, 0), (0, 1), (0, 2), (0, 3), (1, 0), (1, 1), (1, 2), (1, 3)]
            extras_by_unit = [
                pg(kT8, wk_sb, x0_sb, [1]) + pg(qT8[0], wq_sb, xf_sb[0], [1]),
                pg(kT8, wk_sb, x0_sb, [2]) + pg(qT8[0], wq_sb, xf_sb[0], [2]),
                pg(kT8, wk_sb, x0_sb, [3]) + pg(qT8[0], wq_sb, xf_sb[0], [3]),
                pg(qT8[1], wq_sb, xf_sb[1], [0, 1]),
                pg(qT8[1], wq_sb, xf_sb[1], [2, 3]),
                [lambda: oproj(0, 0), lambda: oproj(0, 1)],
                [lambda: oproj(0, 2)],
                [],
            ]
            # PV lags scores/exp by LAG steps so the PV group's exp dependency
            # is already satisfied at dispatch time (the PE sequencer is
            # in-order with a shallow wait queue - a blocked head instruction
            # stalls dispatch of everything behind it)
            NG = len(UNITS) * 8
            LAG = 2
            pts_hist: dict = {}
            pv_cur = None
            oy = None
            for g in range(NG + LAG):
                if g < NG:
                    u, tt = divmod(g, 8)
                    pts_hist[g] = scores_exp(*UNITS[u], tt)
                    if g < 7:
                        vproj(g + 1)
                gp = g - LAG
                if gp >= 0:
                    up, ttp = divmod(gp, 8)
                    fp, hpp = UNITS[up]
                    if ttp == 0:
                        pv_cur = [
                            psv.tile([128, 512], F32, name=f"{P}pv{fp}{hpp}{par}", tag="pv")
                            for par in range(2)
                        ]
                    pv_step(hpp, ttp, pv_cur, pts_hist.pop(gp))
                    if ttp % 2 == 1 and extras_by_unit[up]:
                        extras_by_unit[up].pop(0)()
                    if ttp == 7:
                        finish_unit(fp, hpp, pv_cur, pe_flip=(up == len(UNITS) - 1))
            for m in range(3):
                ps = psb.tile([CO[m][1], S], F32, name=f"{P}oy1{m}", tag="big")
                for sh in range(2):
                    oproj_mm(1, m, ps, sh)
                oproj_fin(1, m, ps, last=True)

    nc.compile()
    return nc


def _get_nc(loop_n: int = 1):
    if loop_n not in _NC_CACHE:
        _NC_CACHE[loop_n] = _build(loop_n)
    return _NC_CACHE[loop_n]


def _pad_heads_cols(wT: np.ndarray) -> np.ndarray:
    """[C, C] (c_in, c_out) -> [C, CP] with each head's 40 cols at h*64."""
    out = np.zeros((C, CP), np.float32)
    out.reshape(C, H, DP)[:, :, :D] = wT.reshape(C, H, D)
    return out


def _prep_inputs(hidden_states, Wq, Wk, Wv, Wo, bo, video_length, k):
    hidden_states = np.asarray(hidden_states, dtype=np.float32)
    B = hidden_states.shape[0]
    assert hidden_states.shape == (B, S, C), hidden_states.shape
    assert B == NCORES * FPC, B
    kf = int(k)
    vl = int(video_length)
    b = B // vl
    assert b == 1, "kernel specialized for batch 1 (b*video_length == B)"

    xT = np.zeros((B, CPAD, S), np.float32)
    xT[:, :C, :] = hidden_states.transpose(0, 2, 1)
    xT = xT.astype(BF)
    wk_p = _pad_heads_cols(np.asarray(Wk, np.float32).T)
    wq_p = _pad_heads_cols(np.asarray(Wq, np.float32).T)
    wkq_p = np.zeros((CPAD, 2 * CP), np.float32)
    wkq_p[:C] = np.concatenate([wk_p, wq_p], axis=1)
    whot_p = np.zeros((CPAD, 256), np.float32)
    whot_p[:C] = np.concatenate([wk_p[:, 0:128], wq_p[:, 0:128]], axis=1)
    wv_p = np.zeros((CPAD, C), np.float32)
    wv_p[:C] = np.asarray(Wv, np.float32).T
    # WoT padded rows: row h*64+j = Wo[:, h*40+j]; pad rows (incl. the ones/
    # denominator row 63) are zero
    wo_p = np.zeros((CP, C), np.float32)
    wo_p.reshape(H, DP, C)[:, :D, :] = np.asarray(Wo, np.float32).T.reshape(H, D, C)
    bo_f = np.zeros(CPAD, np.float32)
    bo_f[:C] = np.asarray(bo, np.float32)

    xt0 = np.ascontiguousarray(xT[kf])
    wkq_b = wkq_p.astype(BF)
    whot_b = whot_p.astype(BF)
    wv_b = wv_p.astype(BF)
    wo_b = wo_p.astype(BF)
    ident_b = np.eye(128, dtype=np.float32).astype(BF)
    in_maps = []
    for c in range(NCORES):
        in_maps.append(
            {
                "xt0": xt0,
                "xtf": np.ascontiguousarray(xT[c * FPC:(c + 1) * FPC]),
                "wkq": wkq_b,
                "whot": whot_b,
                "wvp": wv_b,
                "wo": wo_b,
                "bo": bo_f,
                "ident": ident_b,
            }
        )
    return in_maps


def _run(inputs: dict, loop_n: int = 1):
    global LAST_RESULTS
    nc = _get_nc(loop_n)
    in_maps = _prep_inputs(**inputs)
    last_exc = None
    for _attempt in range(3):
        try:
            res = bass_utils.run_bass_kernel_spmd(nc, in_maps, core_ids=list(range(NCORES)))
            break
        except Exception as e:  # transient NRT/axon device hiccups
            last_exc = e
            import time as _time
            _time.sleep(2.0)
    else:
        raise last_exc
    LAST_RESULTS = res
    B = NCORES * FPC
    y = np.empty((B, S, C), np.float32)
    for c in range(NCORES):
        y[c * FPC:(c + 1) * FPC] = (
            res.results[c]["yt"].astype(np.float32).transpose(0, 2, 1)
        )
    return y


def kernel(hidden_states, Wq, Wk, Wv, Wo, bo, video_length, k):
    return _run(
        dict(
            hidden_states=hidden_states,
            Wq=Wq,
            Wk=Wk,
            Wv=Wv,
            Wo=Wo,
            bo=bo,
            video_length=video_length,
            k=k,
        )
    )
